# revision 1
# baseline (speedup 1.0000x reference)
"""Trainium2 Bass kernel for nn_DynamicMLP (3-layer LIF spiking net, T=16).

Strategy (8 NeuronCores, data-parallel over batch):
  - Shard batch 1024 -> 8 x 128. Replicate weights. Zero cross-core comms.
  - Layout: [batch=128 partitions, hidden on free dim].
  - The LIF current state c lives ENTIRELY in PSUM, scaled by 2^t:
      C_t = sum_{tau<=t} 2^tau * I_tau  ==  2^t * c_t  (bitwise-equivalent to the
      reference's c = 0.5*c + I decay, since powers of 2 are exact).
    Inputs are pre-scaled by 2^t on host (x) / on device (spikes).
  - The output is chaotically sensitive (1e-6 current noise -> 2% output
    error), so matmuls must be fp32-exact. They run as fp16 multi-term splits
    (fp16 x fp16 products are exact in fp32 PSUM accumulation; all stored
    operands kept in fp16 normal range; ~1e-7 residual):
      L0: x = xh + xl exactly (fp16 pair). 2^t*xh@wh -> C0;
          2^(t+11)*xl@wh and 2^t*xh@(wl*2^11) -> C0b (folded at 2^-(t+11)).
      L1/L2: spikes s*2^t are fp16-exact; s_hi@wh -> C and
          (s_hi*2^-11)@(wl*2^11) -> C, same scale, no extra banks.
    Residual error ~2e-8 per current, inside the fp32 matmul-order envelope.
  - Spikes are emitted as fp16 * 2^t and DMA-transposed (xbar) to become the
    next layer's stationary operand. Biases enter via a K=2 matmul row pair
    (rows scaled 2^t and 2^(t-11) for the hi/lo bias split).
"""
import sys

sys.path.insert(0, "/opt/trn_rl_repo")

import numpy as np

import concourse.bacc as bacc
import concourse.tile as tile
from concourse import mybir
from concourse.bass_utils import run_bass_kernel_spmd

dt = mybir.dt
F16 = dt.float16
F32 = dt.float32
Alu = mybir.AluOpType

NCORES = 8
FULL = dict(T=16, IN=2048, H0=1024, H1=1024, OUT=512, BL=128)
EXACT_ORDER = True  # reproduce the reference LIF rounding order exactly

_BUILD_CACHE = {}


def build(T=16, IN=2048, H0=1024, H1=1024, OUT=512, BL=128):
    key = (T, IN, H0, H1, OUT, BL, EXACT_ORDER)
    if key in _BUILD_CACHE:
        return _BUILD_CACHE[key]
    KT0, KT1, KT2 = IN // 128, H0 // 128, H1 // 128
    NCH = 512  # psum bank free-dim (fp32)

    nc = bacc.Bacc("TRN2", target_bir_lowering=False, debug=False, num_devices=NCORES)

    xa_d = nc.dram_tensor("xa", [T, IN, BL], F16, kind="ExternalInput")
    xr_d = nc.dram_tensor("xr", [T, IN, BL], F16, kind="ExternalInput")
    w_d = {}
    for nm, (a, b) in {"w0": (IN, H0), "w1": (H0, H1), "wo": (H1, OUT)}.items():
        w_d[nm + "a"] = nc.dram_tensor(nm + "a", [a, b], F16, kind="ExternalInput")
        w_d[nm + "l"] = nc.dram_tensor(nm + "l", [a, b], F16, kind="ExternalInput")
    b_d = {}
    for nm, h in {"b0": H0, "b1": H1, "b2": OUT}.items():
        b_d[nm] = nc.dram_tensor(nm, [2, h], F16, kind="ExternalInput")
    ones_d = nc.dram_tensor("onesrows", [2, T * 128], F16, kind="ExternalInput")
    id_d = nc.dram_tensor("ident", [128, 128], F16, kind="ExternalInput")
    out_d = nc.dram_tensor("out", [BL, OUT], F32, kind="ExternalOutput")

    with tile.TileContext(nc) as tc:
        with tc.tile_pool(name="w", bufs=1) as wp, \
             tc.tile_pool(name="state", bufs=1) as sp, \
             tc.tile_pool(name="xs", bufs=3) as xp, \
             tc.tile_pool(name="spk", bufs=2) as kp, \
             tc.tile_pool(name="psum", bufs=1, space="PSUM") as pp:

            # ---- resident weights (DMA order = first-use order) ----
            KH = max(KT0 // 2, 1)
            NX0 = KT0 // KH
            w_sb = {}
            for nm, (kt, h) in {"w1": (KT1, H1), "wo": (KT2, OUT)}.items():
                for sfx in ("a", "l"):
                    w_sb[nm + sfx] = wp.tile([128, kt * h], F16, tag=nm + sfx,
                                             name=nm + sfx)
            # w0 in per-chunk tiles so L0 can start after the first chunk lands
            for sfx in ("a", "l"):
                w_sb["w0" + sfx] = [
                    wp.tile([128, KH * H0], F16, tag=f"w0{sfx}{ci}", name=f"w0{sfx}{ci}")
                    for ci in range(NX0)]

            def dma_weights(nm, kt, h):
                for sfx in ("a", "l"):
                    tl = w_sb[nm + sfx]
                    for k in range(kt):
                        if nm == "w0":
                            nc.sync.dma_start(
                                out=tl[k // KH][:, (k % KH) * h:(k % KH + 1) * h],
                                in_=w_d[nm + sfx][k * 128:(k + 1) * 128, :])
                        else:
                            nc.sync.dma_start(out=tl[:, k * h:(k + 1) * h],
                                              in_=w_d[nm + sfx][k * 128:(k + 1) * 128, :])

            b_sb = {}
            for nm, h in {"b0": H0, "b1": H1, "b2": OUT}.items():
                tl = wp.tile([2, h], F16, tag=nm, name=nm)
                nc.sync.dma_start(out=tl[:], in_=b_d[nm][:])
                b_sb[nm] = tl


            # ---- states (single-buffered; DVE program order serializes) ----
            HS = {0: H0, 1: H1, 2: OUT}
            st = {}
            for l in (0, 1, 2):
                for nm in ("v", "u0", "v0", "q"):
                    st[(l, nm)] = sp.tile([128, HS[l]], F32, tag=f"{nm}{l}", name=f"{nm}{l}")
            c021 = sp.tile([128, max(H0, H1)], F32, tag="c021")
            scrA = sp.tile([128, max(H0, H1)], F32, tag="scrA")
            scrB12 = sp.tile([128, H1], F32, tag="scrB12", name="scrB12")
            scrB0b = sp.tile([128, H0], F32, tag="scrB0b", name="scrB0b")
            scrB = {0: sp.tile([128, H0], F32, tag="scrB0", name="scrB0"),
                    1: scrB12, 2: scrB12}
            # psum current accumulators (2^t-scaled)
            C = {0: pp.tile([128, H0], F32, tag="C0", name="C0"),
                 1: pp.tile([128, H1], F32, tag="C1", name="C1"),
                 2: pp.tile([128, OUT], F32, tag="C2", name="C2")}
            C0b = pp.tile([128, H0], F32, tag="C0b", name="C0b")
            accP = pp.tile([128, OUT], F32, tag="accP", name="accP")
            ident = wp.tile([128, 128], F16, tag="ident", name="ident")
            nc.sync.dma_start(out=ident[:], in_=id_d[:])


            # ---- init ----
            for l in (0, 1, 2):
                for nm in ("v", "u0", "v0", "q"):
                    nc.vector.memset(st[(l, nm)][:], 0.0)
            nc.vector.memset(c021[:], 0.021)

            def lif_B(l, t):
                """Release C[l] (+C0b) into scratch on ACT (short queue, and
                the 2^-t scales are exact powers of two -> no rounding)."""
                h = HS[l]
                nc.scalar.mul(scrB[l][:, :h], C[l][:], float(2.0 ** -t))
                if l == 0:
                    nc.scalar.mul(scrB0b[:], C0b[:], float(2.0 ** -(t + 11)))

            def lif_ops(l, t, s_out, last=False):
                """Emit LIF elementwise ops for layer l at step t.

                Consumes C[l] (psum, = 2^t * c_t), states v0/u0/q from step t-1.
                Produces v (=v_t), updates u0/v0/q for t+1, and (if s_out) the
                2^t-scaled fp16 spike tensor.
                """
                h = HS[l]
                v, u0, v0, q = (st[(l, n)] for n in ("v", "u0", "v0", "q"))
                A = scrA[:, :h]
                if EXACT_ORDER:
                    B = scrB[l][:, :h]
                    if not last:
                        # u_t = u0 + ((-0.172*v0) + 0.529*u0)  (reference rounding)
                        nc.scalar.mul(A, v0[:], -0.172)
                        nc.vector.scalar_tensor_tensor(
                            out=A, in0=u0[:], scalar=0.529, in1=A,
                            op0=Alu.mult, op1=Alu.add)
                        nc.vector.tensor_tensor(out=A, in0=u0[:], in1=A, op=Alu.add)
                    # dv = ((q - v0) - u0) + c;  v = v0 + dv   (reference rounding)
                    nc.vector.tensor_tensor(out=v[:], in0=q[:], in1=v0[:],
                                            op=Alu.subtract)
                    nc.vector.tensor_tensor(out=v[:], in0=v[:], in1=u0[:],
                                            op=Alu.subtract)
                    if l == 0:
                        nc.vector.tensor_tensor(out=v[:], in0=v[:], in1=scrB0b[:],
                                                op=Alu.add)
                    nc.vector.tensor_tensor(out=v[:], in0=v[:], in1=B, op=Alu.add)
                    nc.vector.tensor_tensor(out=v[:], in0=v0[:], in1=v[:],
                                            op=Alu.add)
                else:
                    # u_t = 1.529*(u0 - (0.172/1.529)*v0)   (A := u_t)
                    nc.vector.scalar_tensor_tensor(
                        out=A, in0=v0[:], scalar=float(-0.172 / 1.529), in1=u0[:],
                        op0=Alu.mult, op1=Alu.add)
                    nc.vector.tensor_scalar(out=A, in0=A, scalar1=1.529,
                                            scalar2=None, op0=Alu.mult)
                    # v_t = (q - u0) + [2^-(t+11) * C0b] + 2^-t * C
                    nc.vector.tensor_tensor(out=v[:], in0=q[:], in1=u0[:],
                                            op=Alu.subtract)
                    if l == 0:
                        nc.vector.scalar_tensor_tensor(
                            out=v[:], in0=C0b[:], scalar=float(2.0 ** -(t + 11)),
                            in1=v[:], op0=Alu.mult, op1=Alu.add)
                    nc.vector.scalar_tensor_tensor(
                        out=v[:], in0=C[l][:], scalar=float(2.0 ** -t), in1=v[:],
                        op0=Alu.mult, op1=Alu.add)
                # spikes (scale 2^t for l<2; unscaled for l==2) -> fp16
                s_scale = 1.0 if l == 2 else float(2.0 ** t)
                nc.vector.tensor_scalar(out=s_out, in0=v[:], scalar1=0.5,
                                        scalar2=s_scale, op0=Alu.is_gt,
                                        op1=Alu.mult)
                if l == 2:
                    pending_acc.append((t, s_out))
                if last:
                    return
                # u0_{t+1} = u_t + 0.132 * s_t     (unscale s_out)
                nc.vector.scalar_tensor_tensor(
                    out=u0[:], in0=s_out, scalar=float(0.132 / s_scale), in1=A,
                    op0=Alu.mult, op1=Alu.add)
                # v0_{t+1} = v_t with 0.021 where spiked
                nc.scalar.copy(v0[:], v[:])
                nc.vector.copy_predicated(out=v0[:], mask=s_out.bitcast(dt.uint16),
                                          data=c021[:, :h])
                # q_{t+1} = v0^2
                nc.scalar.square(q[:], v0[:])

            def matmuls(l, t, kt, h, lhsA, lhsR, wa, wl, bias, ones2,
                        k_base=0, bias_too=True, kt_total=None):
                """Accumulate 2^t * (x@W + b) into C[l] (+C0b lo-part for l=0).

                l==0: lhsA = 2^t*xh tiles, lhsR = 2^(t+11)*xl tiles.
                      lhsA@wa -> C0; lhsR@wa -> C0b; lhsA@wl(*2^11) -> C0b.
                l>0:  lhsA = 2^t*s_hi tiles, lhsR = 2^(t-11)*s_hi tiles.
                      lhsA@wa -> C; lhsR@wl(*2^11) -> C.
                start=True is emitted per PSUM bank (each n0 chunk) at t==0.
                """
                kt_total = kt_total if kt_total is not None else kt
                for k in range(kt):
                    kg = k_base + k
                    for n0 in range(0, h, NCH):
                        nn = min(NCH, h - n0)
                        first = (t == 0 and kg == 0)
                        last = (t == T - 1 and kg == kt_total - 1)
                        ps = C[l][:, n0:n0 + nn]
                        ra = wa[:, k * h + n0: k * h + n0 + nn]
                        rl = wl[:, k * h + n0: k * h + n0 + nn]
                        la = lhsA[:, k * 128:(k + 1) * 128]
                        lr = lhsR[:, k * 128:(k + 1) * 128]
                        nc.tensor.matmul(ps, la, ra, start=first,
                                         stop=False, skip_group_check=True)
                        if l == 0:
                            psb = C0b[:, n0:n0 + nn]
                            nc.tensor.matmul(psb, lr, ra, start=first,
                                             stop=False, skip_group_check=True)
                            nc.tensor.matmul(psb, la, rl, start=False, stop=last,
                                             skip_group_check=True)
                        else:
                            nc.tensor.matmul(ps, lr, rl, start=False, stop=False,
                                             skip_group_check=True)
                if bias_too:
                    for n0 in range(0, h, NCH):
                        nn = min(NCH, h - n0)
                        nc.tensor.matmul(C[l][:, n0:n0 + nn], ones2[:],
                                         bias[:, n0:n0 + nn], start=False,
                                         stop=(t == T - 1), skip_group_check=True)

            ones2_h = {}
            pending_acc = []

            def flush_acc():
                while pending_acc:
                    ta, s2ap = pending_acc.pop(0)
                    nc.tensor.matmul(accP[:], ident[:], s2ap, start=(ta == 0),
                                     stop=(ta == T - 1), skip_group_check=True)

            x_pre = {}

            def load_x(t):
                ones2 = xp.tile([2, 128], F16, tag="ones2", name=f"ones2_t{t}")
                nc.sync.dma_start(out=ones2[:], in_=ones_d[:, t * 128:(t + 1) * 128])
                ones2_h[t] = ones2
                tiles = []
                for ci in range(NX0):
                    xa_t = xp.tile([128, KH * BL], F16, tag="xa", name=f"xa_t{t}_{ci}")
                    xr_t = xp.tile([128, KH * BL], F16, tag="xr", name=f"xr_t{t}_{ci}")
                    ks = ci * KH * 128
                    nc.sync.dma_start(
                        out=xa_t[:].rearrange("p (k b) -> p k b", b=BL),
                        in_=xa_d[t:t + 1, ks:ks + KH * 128].rearrange(
                            "o (k p) b -> p (o k) b", p=128))
                    nc.sync.dma_start(
                        out=xr_t[:].rearrange("p (k b) -> p k b", b=BL),
                        in_=xr_d[t:t + 1, ks:ks + KH * 128].rearrange(
                            "o (k p) b -> p (o k) b", p=128))
                    tiles.append((xa_t, xr_t))
                x_pre[t] = tiles

            def emit_L0(t, cis=None):
                if t not in x_pre:
                    load_x(t)
                tiles = x_pre[t]
                if cis is None or 1 in cis:
                    x_pre.pop(t, None)
                ones2 = ones2_h[t]
                for ci in (cis if cis is not None else range(NX0)):
                    xa_t, xr_t = tiles[ci]
                    matmuls(0, t, KH, H0, xa_t[:], xr_t[:],
                            w_sb["w0a"][ci][:], w_sb["w0l"][ci][:],
                            b_sb["b0"], ones2[:], k_base=ci * KH,
                            bias_too=(ci == NX0 - 1), kt_total=KT0)

            def emit_rest(t, filler=None):
                flush_acc()
                ones2 = ones2_h[t]
                s0 = kp.tile([128, H0], F16, tag="sPre", name=f"s0_t{t}")
                lif_ops(0, t, s0[:], last=(t == T - 1))  # B0 emitted by caller
                s0T = kp.tile([128, H0], F16, tag="sT", name=f"s0T_t{t}")
                nc.sync.dma_start_transpose(
                    out=s0T[:].rearrange("p (k b) -> p k b", b=128), in_=s0[:])
                s0L = kp.tile([128, H0], F16, tag="sL", name=f"s0L_t{t}", bufs=2)
                nc.vector.tensor_scalar(out=s0L[:], in0=s0T[:],
                                        scalar1=float(2.0 ** -11), scalar2=None,
                                        op0=Alu.mult)
                matmuls(1, t, KT1, H1, s0T[:], s0L[:], w_sb["w1a"], w_sb["w1l"],
                        b_sb["b1"], ones2[:])
                lif_B(1, t)
                if filler is not None:
                    filler()
                s1 = kp.tile([128, H1], F16, tag="sPre", name=f"s1_t{t}")
                lif_ops(1, t, s1[:], last=(t == T - 1))
                s1T = kp.tile([128, H1], F16, tag="sT", name=f"s1T_t{t}")
                nc.sync.dma_start_transpose(
                    out=s1T[:].rearrange("p (k b) -> p k b", b=128), in_=s1[:])
                s1L = kp.tile([128, H1], F16, tag="sL", name=f"s1L_t{t}", bufs=2)
                nc.vector.tensor_scalar(out=s1L[:], in0=s1T[:],
                                        scalar1=float(2.0 ** -11), scalar2=None,
                                        op0=Alu.mult)
                matmuls(2, t, KT2, OUT, s1T[:], s1L[:], w_sb["woa"], w_sb["wol"],
                        b_sb["b2"], ones2[:])
                lif_B(2, t)
                s2 = kp.tile([128, OUT], F16, tag="s2", name=f"s2_t{t}", bufs=2)
                lif_ops(2, t, s2[:], last=(t == T - 1))
                ones2_h.pop(t, None)

            # preamble DMAs in first-use order: x(0) first, then weights
            load_x(0)
            for ci in range(NX0):
                for sfx in ("a", "l"):
                    tl = w_sb["w0" + sfx][ci]
                    for kk in range(KH):
                        k = ci * KH + kk
                        nc.sync.dma_start(out=tl[:, kk * H0:(kk + 1) * H0],
                                          in_=w_d["w0" + sfx][k * 128:(k + 1) * 128, :])
            dma_weights("w1", KT1, H1)
            dma_weights("wo", KT2, OUT)

            # 1-step layer skew: PE gets L0(t+1) while the t chain drains
            for t in range(T):
                if t >= 1:
                    lif_B(0, t - 1)       # free C0/C0b for step t's matmuls
                emit_L0(t, cis=(0,))
                if t + 1 < T:
                    load_x(t + 1)
                if t >= 1:
                    emit_rest(t - 1, filler=lambda tt=t: emit_L0(tt, cis=(1,)))
                else:
                    emit_L0(t, cis=(1,))
            lif_B(0, T - 1)
            emit_rest(T - 1)

            flush_acc()
            accS = sp.tile([128, OUT], F32, tag="accS", name="accS")
            nc.vector.tensor_copy(out=accS[:], in_=accP[:])
            nc.sync.dma_start(out=out_d[:], in_=accS[:])

    nc.compile()
    _BUILD_CACHE[key] = nc
    return nc


def _split_f16(a32, lo_scale=2048.0):
    """a32 ~ hi + lo*2^-11 with hi = fp16(a32), lo = fp16((a32-hi)*2^11)."""
    hi = a32.astype(np.float16)
    lo = ((a32 - hi.astype(np.float32)) * np.float32(lo_scale)).astype(np.float16)
    return hi, lo


def prep_inputs(in_pop_spikes, W0, b0, W1, b1, Wout, bout,
                T=16, BL=128, ncores=NCORES):
    """Host-side prep: transpose/scale/split x, split weights; 8 in_maps."""
    x = np.ascontiguousarray(np.transpose(np.asarray(in_pop_spikes, np.float32),
                                          (2, 1, 0)))  # [T, IN, B]
    scale = (2.0 ** np.arange(T, dtype=np.float32)).reshape(T, 1, 1)
    xh32 = x.astype(np.float16).astype(np.float32)
    xa = (xh32 * scale).astype(np.float16)                 # exact 2^t * fp16(x)
    xr = ((x - xh32) * (scale * np.float32(2048.0))).astype(np.float16)
    # ^ 2^(t+11) * xl, fp16 (xl itself is the exact fp32 residual)

    com = {}
    for nm, W in (("w0", W0), ("w1", W1), ("wo", Wout)):
        WT = np.ascontiguousarray(np.asarray(W, np.float32).T)
        com[nm + "a"], com[nm + "l"] = _split_f16(WT)
    for nm, b in (("b0", b0), ("b1", b1), ("b2", bout)):
        hi, lo = _split_f16(np.asarray(b, np.float32))
        com[nm] = np.stack([hi, lo])

    T_ = T
    onesrows = np.zeros((2, T_ * 128), np.float16)
    for t in range(T_):
        onesrows[0, t * 128:(t + 1) * 128] = np.float16(2.0 ** t)
        onesrows[1, t * 128:(t + 1) * 128] = np.float16(2.0 ** (t - 11))
    com["onesrows"] = onesrows
    com["ident"] = np.eye(128, dtype=np.float16)

    in_maps = []
    for c in range(ncores):
        m = dict(com)
        m["xa"] = np.ascontiguousarray(xa[:, :, c * BL:(c + 1) * BL])
        m["xr"] = np.ascontiguousarray(xr[:, :, c * BL:(c + 1) * BL])
        in_maps.append(m)
    return in_maps


def kernel(in_pop_spikes, W0, b0, W1, b1, Wout, bout, batch_size, _trace=False):
    T = in_pop_spikes.shape[2]
    nc = build(**FULL)
    in_maps = prep_inputs(in_pop_spikes, W0, b0, W1, b1, Wout, bout, T=T)
    res = run_bass_kernel_spmd(nc, in_maps, core_ids=list(range(NCORES)),
                               trace=_trace)
    out = np.concatenate([r["out"] for r in res.results], axis=0)
    out = (out / np.float32(T)).astype(np.float32)
    if _trace:
        kernel._last_results = res
    return out



# revision 42
# speedup vs baseline: 1.0779x; 1.0779x over previous
"""Trainium2 Bass kernel for nn_DynamicMLP (3-layer LIF spiking net, T=16).

Strategy (8 NeuronCores, data-parallel over batch):
  - Shard batch 1024 -> 8 x 128. Replicate weights. Zero cross-core comms.
  - Layout: [batch=128 partitions, hidden on free dim].
  - LIF current state c lives in PSUM scaled by 2^t: C_t = sum 2^tau I_tau.
  - L0 (x @ W0): fp16 multi-term split, fp32-exact to ~2^-22 (the network is
    chaotic: >=20 significant bits needed on BOTH operands; measured).
      xh@wh -> C0;  2^11*xl@wh and xh@(wl*2^11) -> C0b (folded at 2^-(t+11)).
  - L1/L2 (spikes @ W): W decomposed into 6 signed radix-16 digits stored as
    exact fp8e4 planes; spikes (exact powers of two) stored as fp8e5 planes
    at 3 scales {2^t, 2^(t-8), 2^(t-16)}. Matmuls run as fp8 DoubleRow pairs
    (2 digit products per instruction at 0.5 cycles/row): 25% fewer PE cycles
    than the fp16 hi/lo split, ~21.5-bit effective weights (verified exact
    digit reconstruction on device).
  - Biases: one -b*2^e matmul injected into each C group at t=0; the +2b
    constant enters through the fused v-update (c_t = C*2^-t - b*2^-t + 2b).
    This removes all per-step bias matmuls.
  - Elementwise LIF updates are fused (v_t = q - u0 + c) and spread across
    DVE / ACT / GpSimd(Pool) so no single engine shadows the PE.
  - Output accumulation on Pool (acc += v>vth), not the PE.
"""
import sys

sys.path.insert(0, "/opt/trn_rl_repo")

import numpy as np
import ml_dtypes

import concourse.bacc as bacc
import concourse.tile as tile
from concourse import mybir
from concourse.bass_utils import run_bass_kernel_spmd

dt = mybir.dt
F16 = dt.float16
F32 = dt.float32
E4 = dt.float8e4
E5 = dt.float8e5
Alu = mybir.AluOpType
DR = mybir.MatmulPerfMode.DoubleRow

NCORES = 8
FULL = dict(T=16, IN=2048, H0=1024, H1=1024, OUT=512, BL=128)
NDIG = 6
EW = 4          # weight exponent for L1/L2: W*2^EW in (-0.5, 0.5]
WFOLD = [4, 0, 4, 0, 4, 8]        # digit i stored as d_i * 2^-WFOLD[i]
KFOLD = [0, -8, -8, -16, -16, -16]  # ifmap (spike) plane scale exponents
# plane order in the sP tile: [s*2^t, s*2^(t-8), s*2^(t-16), s*2^(t-16)]
PLANE_OF_PAIR = [(0, 1), (1, 2), (2, 3)]  # pairs (d1,d2),(d3,d4),(d5,d6)

_BUILD_CACHE = {}


def build(T=16, IN=2048, H0=1024, H1=1024, OUT=512, BL=128):
    key = (T, IN, H0, H1, OUT, BL)
    if key in _BUILD_CACHE:
        return _BUILD_CACHE[key]
    KT0, KT1, KT2 = IN // 128, H0 // 128, H1 // 128
    NCH = 512  # psum bank free-dim (fp32)

    nc = bacc.Bacc("TRN2", target_bir_lowering=False, debug=False, num_devices=NCORES)

    # xa/xr interleaved per 128-row group: [T, IN/128, 128, 2, BL] — keeps
    # dram runs at 512B (no small-transfer DMA penalty) and halves DMA count
    xz_d = nc.dram_tensor("xz", [T, IN * 2, BL], F16, kind="ExternalInput")
    w0a_d = nc.dram_tensor("w0a", [IN, H0], F16, kind="ExternalInput")
    w0l_d = nc.dram_tensor("w0l", [IN, H0], F16, kind="ExternalInput")
    w1d_d = nc.dram_tensor("w1d", [H0, NDIG * H1], E4, kind="ExternalInput")
    woa_d = nc.dram_tensor("woa", [H1, OUT], F16, kind="ExternalInput")
    wol_d = nc.dram_tensor("wol", [H1, OUT], F16, kind="ExternalInput")
    br_d = {}
    for nm, h in (("br0", H0), ("br1", H1), ("br2", OUT)):
        br_d[nm] = nc.dram_tensor(nm, [2, h], F16, kind="ExternalInput")
    cst_d = {nm: nc.dram_tensor(nm, [2, 128], F16, kind="ExternalInput")
             for nm in ("cpos", "cneg0", "cneg1", "cneg2")}
    out_d = nc.dram_tensor("out", [BL, OUT], F32, kind="ExternalOutput")

    with tile.TileContext(nc) as tc:
        with tc.tile_pool(name="w", bufs=1) as wp, \
             tc.tile_pool(name="state", bufs=1) as sp, \
             tc.tile_pool(name="xs", bufs=1) as xp, \
             tc.tile_pool(name="spk", bufs=1) as kp, \
             tc.tile_pool(name="psum", bufs=1, space="PSUM") as pp:

            # ---- resident weights (DMA order = first-use order) ----
            KH = KT0 // 2          # w0 split in 2 chunk-tiles for skew filler
            NX0 = KT0 // KH
            w_sb = {}
            for sfx in ("a", "l"):
                w_sb["w0" + sfx] = [
                    wp.tile([128, KH * H0], F16, tag=f"w0{sfx}{ci}", name=f"w0{sfx}{ci}")
                    for ci in range(NX0)]
            w1d = wp.tile([128, KT1 * NDIG * H1], E4, tag="w1d", name="w1d")
            # L2 stays on the fp16 hi/lo scheme (cheaper SBUF than digits)
            woa = wp.tile([128, KT2 * OUT], F16, tag="woa", name="woa")
            wol = wp.tile([128, KT2 * OUT], F16, tag="wol", name="wol")

            b_sb = {}
            for nm, h in (("br0", H0), ("br1", H1), ("br2", OUT)):
                tl = wp.tile([2, h], F16, tag=nm, name=nm)
                b_sb[nm] = tl
            # constant 2-row lhsT columns for bias matmuls
            EL = {0: 0, 1: EW, 2: 0}   # per-layer PSUM weight exponent
            cneg = {}   # -2^e rows for the t=0 injection
            cpos = wp.tile([2, 128], F16, tag="cpos", name="cpos")
            for l in (0, 1, 2):
                cneg[l] = wp.tile([2, 128], F16, tag=f"cneg{l}", name=f"cneg{l}")

            # ---- states ----
            HS = {0: H0, 1: H1, 2: OUT}
            st = {}
            for l in (0, 1, 2):
                for nm in ("v0", "u0"):
                    st[(l, nm)] = sp.tile([128, HS[l]], F32, tag=f"{nm}{l}",
                                          name=f"{nm}{l}")
            # transient v/A/U: layer 0 gets its own so the step-t release does
            # not serialize behind the full step t-1 chain
            vT = {0: sp.tile([128, H0], F32, tag="vT0", name="vT0"),
                  1: sp.tile([128, max(H1, OUT)], F32, tag="vT12", name="vT12")}
            vT[2] = vT[1]
            A_ = {0: sp.tile([128, H0], F32, tag="A0", name="A0"),
                  1: sp.tile([128, H1], F32, tag="A1", name="A1"),
                  2: sp.tile([128, OUT], F32, tag="A2", name="A2")}
            U_ = A_  # disjoint lifetimes: A dies at release, U born at post
            c021 = sp.tile([128, max(H0, H1)], F32, tag="c021")
            B2b = {0: sp.tile([128, H0], F32, tag="B2b0", name="B2b0"),
                   1: sp.tile([128, H1], F32, tag="B2b1", name="B2b1"),
                   2: sp.tile([128, OUT], F32, tag="B2b2", name="B2b2")}
            acc = sp.tile([128, OUT], F32, tag="acc", name="acc")
            # psum current accumulators (2^t-scaled)
            C = {0: pp.tile([128, H0], F32, tag="C0", name="C0"),
                 1: pp.tile([128, H1], F32, tag="C1", name="C1"),
                 2: pp.tile([128, OUT], F32, tag="C2", name="C2")}
            C0b = pp.tile([128, H0], F32, tag="C0b", name="C0b")
            pB = pp.tile([128, NCH], F32, tag="pB", name="pB")  # 1 bank, preamble only

            # ---- preamble ----
            # x(0) first so L0 can start immediately
            x_pre = {}

            def load_x(t, cis=None):
                tiles = x_pre.setdefault(t, {})
                for ci in (cis if cis is not None else range(NX0)):
                    if ci in tiles:
                        continue
                    # [128, k, 2, BL]: xa plane 0, xr plane 1, per k-chunk
                    xz_t = xp.tile([128, KH * 2 * BL], F16, tag="xz",
                                   name=f"xz_t{t}_{ci}")
                    ks = ci * KH * 2 * 128
                    nc.sync.dma_start(
                        out=xz_t[:].rearrange("p (k two b) -> p k two b",
                                              two=2, b=BL),
                        in_=xz_d[t:t + 1, ks:ks + KH * 2 * 128].rearrange(
                            "o (k p two) b -> p (o k) two b", p=128, two=2))
                    tiles[ci] = xz_t

            def dma_w0(ci):
                for kk in range(KH):
                    k = ci * KH + kk
                    for sfx in ("a", "l"):
                        tl = w_sb["w0" + sfx][ci]
                        wd = w0a_d if sfx == "a" else w0l_d
                        nc.sync.dma_start(out=tl[:, kk * H0:(kk + 1) * H0],
                                          in_=wd[k * 128:(k + 1) * 128, :])

            def dma_w1d(ks):
                for k in ks:
                    nc.sync.dma_start(
                        out=w1d[:, k * NDIG * H1:(k + 1) * NDIG * H1],
                        in_=w1d_d[k * 128:(k + 1) * 128, :])

            def dma_wo():
                for k in range(KT2):
                    nc.sync.dma_start(out=woa[:, k * OUT:(k + 1) * OUT],
                                      in_=woa_d[k * 128:(k + 1) * 128, :])
                    nc.sync.dma_start(out=wol[:, k * OUT:(k + 1) * OUT],
                                      in_=wol_d[k * 128:(k + 1) * 128, :])

            # just-in-time DMA order (single serialized DMA resource):
            nc.sync.dma_start(out=cpos[:], in_=cst_d["cpos"][:])
            for l in (0, 1, 2):
                nc.sync.dma_start(out=cneg[l][:], in_=cst_d[f"cneg{l}"][:])
            load_x(0, cis=(0,))
            dma_w0(0)
            for nm in ("br0", "br1", "br2"):
                nc.sync.dma_start(out=b_sb[nm][:], in_=br_d[nm][:])
            load_x(0, cis=(1,))
            dma_w0(1)

            # init states + consts
            for l in (0, 1, 2):
                for nm in ("v0", "u0"):
                    nc.vector.memset(st[(l, nm)][:], 0.0)
            nc.vector.memset(c021[:], 0.021)
            nc.vector.memset(acc[:], 0.0)


            bias_of = {0: "br0", 1: "br1", 2: "br2"}

            def build_B2b():
                # B2b_l = 2*b broadcast, built once on PE via pB (1 bank)
                for l in (0, 1, 2):
                    h = HS[l]
                    for n0 in range(0, h, NCH):
                        nn = min(NCH, h - n0)
                        nc.tensor.matmul(pB[:, :nn], cpos[:],
                                         b_sb[bias_of[l]][:, n0:n0 + nn],
                                         start=True, stop=True,
                                         skip_group_check=True)
                        nc.scalar.copy(B2b[l][:, n0:n0 + nn], pB[:, :nn])

            # ---- L0 matmuls (fp16 3-term, bias injected at t=0) ----
            def emit_L0(t, cis):
                load_x(t, cis=cis)
                tiles = x_pre[t]
                for ci in cis:
                    xz_t = tiles.pop(ci)
                    if not tiles:
                        x_pre.pop(t, None)
                    wa = w_sb["w0a"][ci]
                    wl = w_sb["w0l"][ci]
                    for k in range(KH):
                        kg = ci * KH + k
                        la = xz_t[:, (2 * k) * BL:(2 * k + 1) * BL]
                        lr = xz_t[:, (2 * k + 1) * BL:(2 * k + 2) * BL]
                        for n0 in range(0, H0, NCH):
                            first = (t == 0 and kg == 0)
                            last = (t == T - 1 and kg == KT0 - 1)
                            ra = wa[:, k * H0 + n0:k * H0 + n0 + NCH]
                            rl = wl[:, k * H0 + n0:k * H0 + n0 + NCH]
                            nc.tensor.matmul(C[0][:, n0:n0 + NCH], la, ra,
                                             start=first, stop=False,
                                             skip_group_check=True)
                            psb = C0b[:, n0:n0 + NCH]
                            nc.tensor.matmul(psb, lr, ra, start=first, stop=False,
                                             skip_group_check=True)
                            nc.tensor.matmul(psb, la, rl, start=False, stop=last,
                                             skip_group_check=True)
                    if t == 0 and ci == NX0 - 1:
                        for n0 in range(0, H0, NCH):
                            nc.tensor.matmul(C[0][:, n0:n0 + NCH], cneg[0][:],
                                             b_sb["br0"][:, n0:n0 + NCH],
                                             start=False, stop=(T == 1),
                                             skip_group_check=True)

            # ---- fp8 DoubleRow digit matmuls for L1/L2 ----
            def emit_dr(l, t, kt, h, sP, hin, wd):
                """sP: flat [128, 4*hin] e5m2 plane tile; wd: flat digit tile."""
                for k in range(kt):
                    for n0 in range(0, h, NCH):
                        nn = min(NCH, h - n0)
                        for pi, (pa, pb_) in enumerate(PLANE_OF_PAIR):
                            first = (t == 0 and k == 0 and pi == 0)
                            last = (t == T - 1 and k == kt - 1 and pi == 2)
                            lhs = sP[:, pa * hin:(pa + 2) * hin].rearrange(
                                "p (two h) -> p two h", two=2)[
                                :, :, k * 128:(k + 1) * 128]
                            base = (k * NDIG + 2 * pi) * h
                            rhs = wd[:, base:base + 2 * h].rearrange(
                                "p (two h) -> p two h", two=2)[:, :, n0:n0 + nn]
                            nc.tensor.matmul(
                                C[l][:, n0:n0 + nn], lhs, rhs,
                                start=first, stop=last, perf_mode=DR,
                                skip_group_check=True)
                if t == 0:
                    for n0 in range(0, h, NCH):
                        nn = min(NCH, h - n0)
                        nc.tensor.matmul(C[l][:, n0:n0 + nn], cneg[l][:],
                                         b_sb[bias_of[l]][:, n0:n0 + nn],
                                         start=False, stop=False,
                                         skip_group_check=True)

            # ---- fused LIF elementwise ----
            def lif_pre(l, t):
                """Off-path: A = v0*v0 - u0 + B2b (ACT square + 2 DVE adds)."""
                h = HS[l]
                A = A_[l][:, :h]
                v0, u0 = st[(l, "v0")], st[(l, "u0")]
                nc.scalar.square(A, v0[:])
                nc.vector.tensor_tensor(out=A, in0=A, in1=u0[:], op=Alu.subtract)
                nc.vector.tensor_tensor(out=A, in0=A, in1=B2b[l][:], op=Alu.add)

            def lif_release(l, t):
                """DVE part reading PSUM: v = C*2^(-t-e) + A (+ C0b part)."""
                h = HS[l]
                v = vT[l][:, :h]
                nc.vector.scalar_tensor_tensor(
                    out=v, in0=C[l][:], scalar=float(2.0 ** (-t - EL[l])),
                    in1=A_[l][:, :h], op0=Alu.mult, op1=Alu.add)
                if l == 0:
                    nc.vector.scalar_tensor_tensor(
                        out=v, in0=C0b[:], scalar=float(2.0 ** -(t + 11)), in1=v,
                        op0=Alu.mult, op1=Alu.add)

            def lif_post(l, t, s_out, last):
                """Spike + state updates for step t+1."""
                h = HS[l]
                v = vT[l][:, :h]
                v0, u0 = st[(l, "v0")], st[(l, "u0")]
                s_scale = 1.0 if l == 2 else float(2.0 ** t)
                if l == 2:
                    # acc += (v > vth) on Pool
                    nc.vector.scalar_tensor_tensor(
                        out=acc[:], in0=v, scalar=0.5, in1=acc[:],
                        op0=Alu.is_gt, op1=Alu.add)
                    if last:
                        return
                nc.vector.tensor_scalar(out=s_out, in0=v, scalar1=0.5,
                                        scalar2=s_scale, op0=Alu.is_gt,
                                        op1=Alu.mult)
                if last:
                    return
                # u* = 1.529*u0 - 0.172*v0   (Pool stt, ACT scale)
                U = U_[l][:, :h]
                nc.vector.scalar_tensor_tensor(
                    out=U, in0=v0[:], scalar=float(-0.172 / 1.529), in1=u0[:],
                    op0=Alu.mult, op1=Alu.add)
                nc.scalar.mul(U, U, 1.529)
                # u0' = u* + 0.132 * s
                nc.vector.scalar_tensor_tensor(
                    out=u0[:], in0=s_out, scalar=float(0.132 / s_scale), in1=U,
                    op0=Alu.mult, op1=Alu.add)
                # v0' = v with 0.021 where spiked
                nc.scalar.copy(v0[:], v)
                nc.vector.copy_predicated(out=v0[:], mask=s_out.bitcast(dt.uint16),
                                          data=c021[:, :h])

            def make_planes(l, t, sT, hin):
                """4 fp8e5 scaled copies of the transposed spikes (flat tile)."""
                sP = kp.tile([128, 4 * hin], E5, tag=f"sP{l}", name=f"sP{l}_t{t}")
                nc.scalar.copy(sP[:, 0:hin], sT[:])
                nc.vector.tensor_scalar(out=sP[:, hin:2 * hin], in0=sT[:],
                                        scalar1=float(2.0 ** -8), scalar2=None,
                                        op0=Alu.mult)
                nc.scalar.mul(sP[:, 2 * hin:3 * hin], sT[:], float(2.0 ** -16))
                nc.vector.tensor_scalar(out=sP[:, 3 * hin:4 * hin], in0=sT[:],
                                        scalar1=float(2.0 ** -16), scalar2=None,
                                        op0=Alu.mult)
                return sP

            def emit_rest(t, filler=None):
                last = (t == T - 1)
                # --- layer 0 spike + states (release already emitted) ---
                s0 = kp.tile([128, H0], F16, tag="sPre", name=f"s0_t{t}")
                lif_post(0, t, s0[:], last)
                if not last:
                    lif_pre(0, t + 1)   # off critical path: only states needed
                s0T = kp.tile([128, H0], F16, tag="sT", name=f"s0T_t{t}")
                nc.sync.dma_start_transpose(
                    out=s0T[:].rearrange("p (k b) -> p k b", b=128), in_=s0[:])
                sP0 = make_planes(1, t, s0T, H0)
                emit_dr(1, t, KT1, H1, sP0, H0, w1d)
                lif_release(1, t)
                if filler is not None:
                    filler()
                s1 = kp.tile([128, H1], F16, tag="sPre1", name=f"s1_t{t}")
                lif_post(1, t, s1[:], last)
                if not last:
                    lif_pre(1, t + 1)
                s1T = kp.tile([128, H1], F16, tag="sT1", name=f"s1T_t{t}")
                nc.sync.dma_start_transpose(
                    out=s1T[:].rearrange("p (k b) -> p k b", b=128), in_=s1[:])
                s1L = kp.tile([128, H1], F16, tag="sL1", name=f"s1L_t{t}")
                nc.vector.tensor_scalar(out=s1L[:], in0=s1T[:],
                                        scalar1=float(2.0 ** -11), scalar2=None,
                                        op0=Alu.mult)
                # L2: fp16 hi/lo 2-term
                for k in range(KT2):
                    la = s1T[:, k * 128:(k + 1) * 128]
                    ll = s1L[:, k * 128:(k + 1) * 128]
                    first = (t == 0 and k == 0)
                    lastm = (t == T - 1 and k == KT2 - 1)
                    nc.tensor.matmul(C[2][:], la, woa[:, k * OUT:(k + 1) * OUT],
                                     start=first, stop=False,
                                     skip_group_check=True)
                    nc.tensor.matmul(C[2][:], ll, wol[:, k * OUT:(k + 1) * OUT],
                                     start=False, stop=lastm,
                                     skip_group_check=True)
                if t == 0:
                    nc.tensor.matmul(C[2][:], cneg[2][:], b_sb["br2"][:],
                                     start=False, stop=False,
                                     skip_group_check=True)
                lif_release(2, t)
                if last:
                    lif_post(2, t, None, last)
                else:
                    s2 = kp.tile([128, OUT], F16, tag="s2", name=f"s2_t{t}")
                    lif_post(2, t, s2[:], last)
                    lif_pre(2, t + 1)

            # ---- main loop: 1-step layer skew ----
            for t in range(T):
                if t >= 1:
                    lif_release(0, t - 1)   # frees C0/C0b for step t's matmuls
                emit_L0(t, cis=(0,))
                if t == 0:
                    # remaining weight DMAs, just-in-time behind x(1)
                    dma_w1d(range(0, 4))
                    load_x(1, cis=(0,))
                    dma_w1d(range(4, KT1))
                    dma_wo()
                    load_x(1, cis=(1,))
                    build_B2b()
                    for l in (0, 1, 2):
                        lif_pre(l, 0)
                    emit_L0(0, cis=(1,))
                else:
                    emit_rest(t - 1, filler=lambda tt=t: emit_L0(tt, cis=(1,)))
                    if t + 1 < T:
                        load_x(t + 1)
            lif_release(0, T - 1)
            emit_rest(T - 1)

            nc.sync.dma_start(out=out_d[:], in_=acc[:])

    nc.compile()
    _BUILD_CACHE[key] = nc
    return nc


def _split_f16(a32, lo_scale=2048.0):
    hi = a32.astype(np.float16)
    lo = ((a32 - hi.astype(np.float32)) * np.float32(lo_scale)).astype(np.float16)
    return hi, lo


def _digit_planes(WT, ndig=NDIG, ew=EW):
    """WT [in,out] fp32 -> [in, ndig*out] fp8e4 digit planes (folded)."""
    r = WT.astype(np.float64) * (2.0 ** ew)
    assert np.max(np.abs(r)) <= 0.5, "weight exponent EW too small"
    planes = []
    for i in range(1, ndig + 1):
        di = np.rint(r * 16.0 ** i)
        di = np.clip(di, -4, 4) if i == ndig else np.clip(di, -8, 8)
        r = r - di * 16.0 ** -i
        planes.append(di * 2.0 ** -WFOLD[i - 1])
    out = np.concatenate(planes, axis=1).astype(ml_dtypes.float8_e4m3fn)
    assert np.all(out.astype(np.float64) == np.concatenate(planes, axis=1)), \
        "digit planes not exact in fp8e4"
    return out


def prep_inputs(in_pop_spikes, W0, b0, W1, b1, Wout, bout,
                T=16, BL=128, ncores=NCORES):
    x = np.ascontiguousarray(np.transpose(np.asarray(in_pop_spikes, np.float32),
                                          (2, 1, 0)))  # [T, IN, B]
    B = x.shape[2]
    IN = x.shape[1]
    scale = (2.0 ** np.arange(T, dtype=np.float32)).reshape(T, 1, 1)
    xh32 = x.astype(np.float16).astype(np.float32)
    xa = (xh32 * scale).astype(np.float16)
    xr = ((x - xh32) * (scale * np.float32(2048.0))).astype(np.float16)
    # interleave per 128-row group: [T, k, p, {xa,xr}, B] -> [T, IN*2, B]
    xz = np.stack([xa.reshape(T, IN // 128, 128, B),
                   xr.reshape(T, IN // 128, 128, B)], axis=3)
    xz = np.ascontiguousarray(xz.reshape(T, IN * 2, B))

    com = {}
    W0T = np.ascontiguousarray(np.asarray(W0, np.float32).T)
    com["w0a"], com["w0l"] = _split_f16(W0T)
    com["w1d"] = _digit_planes(np.ascontiguousarray(np.asarray(W1, np.float32).T))
    WoT = np.ascontiguousarray(np.asarray(Wout, np.float32).T)
    com["woa"], com["wol"] = _split_f16(WoT)
    for nm, b in (("br0", b0), ("br1", b1), ("br2", bout)):
        hi, lo = _split_f16(np.asarray(b, np.float32))
        com[nm] = np.stack([hi, lo])
    EL = {0: 0.0, 1: float(2.0 ** EW), 2: 0.0}
    com["cpos"] = np.stack([np.full(128, 2.0, np.float16),
                            np.full(128, 2.0 / 2048.0, np.float16)])
    for l in (0, 1, 2):
        e = 2.0 ** EW if l == 1 else 1.0
        com[f"cneg{l}"] = np.stack([np.full(128, -e, np.float16),
                                    np.full(128, -e / 2048.0, np.float16)])

    in_maps = []
    for c in range(ncores):
        m = dict(com)
        m["xz"] = np.ascontiguousarray(xz[:, :, c * BL:(c + 1) * BL])
        in_maps.append(m)
    return in_maps


def kernel(in_pop_spikes, W0, b0, W1, b1, Wout, bout, batch_size, _trace=False):
    T = in_pop_spikes.shape[2]
    nc = build(**FULL)
    in_maps = prep_inputs(in_pop_spikes, W0, b0, W1, b1, Wout, bout, T=T)
    res = run_bass_kernel_spmd(nc, in_maps, core_ids=list(range(NCORES)),
                               trace=_trace)
    out = np.concatenate([r["out"] for r in res.results], axis=0)
    out = (out / np.float32(T)).astype(np.float32)
    if _trace:
        kernel._last_results = res
    return out


# revision 43
# speedup vs baseline: 1.1247x; 1.0434x over previous
"""Trainium2 Bass kernel for nn_DynamicMLP (3-layer LIF spiking net, T=16).

Strategy (8 NeuronCores, data-parallel over batch):
  - Shard batch 1024 -> 8 x 128. Replicate weights. Zero cross-core comms.
  - TRANSPOSED layout: [hidden on partitions (128-chunks on free), batch on
    free dim]. Weights are the stationary matmul operand, activations the
    moving one; every layer's spikes come out exactly in the next layer's
    moving-operand layout, so there are NO DMA transposes anywhere.
  - LIF current state c lives in PSUM scaled by 2^t: C_t = sum 2^tau I_tau.
  - L0 (x @ W0): fp16 multi-term split, fp32-exact to ~2^-22 (the network is
    chaotic: >=20 significant bits needed on BOTH operands; measured):
      w0h.T@xh -> C0;  w0h.T@(2^11*xl) and (w0l*2^11).T@xh -> C0b,
    folded at 2^-(t+11) on release.
  - L1 (spikes @ W1): W1 decomposed into 6 signed radix-16 digits stored as
    exact fp8e4 planes (stationary); spikes as fp8e5 planes at 3 scales
    {2^t, 2^(t-8), 2^(t-16)} (moving). Matmuls run as fp8 DoubleRow pairs
    (2 digit products per instr at 0.5 cycles/row): 25% fewer PE cycles than
    the fp16 hi/lo split, ~21.5-bit effective weights (verified on device).
  - L2: fp16 hi/lo 2-term (digit planes for it don't fit SBUF).
  - Biases: one -b*2^e matmul injected into each C group at t=0; the +2b
    constant enters through the fused v-update (c_t = C*2^-t - b*2^-t + 2b).
    No per-step bias matmuls.
  - Fused LIF elementwise (v_t = v0^2 - u0 + c) split across DVE + ACT; the
    per-layer A = v0^2 - u0 + B2b is precomputed one step early off the
    critical path. Output accumulation (acc += v>vth) on DVE, not the PE.
"""
import sys

sys.path.insert(0, "/opt/trn_rl_repo")

import numpy as np
import ml_dtypes

import concourse.bacc as bacc
import concourse.tile as tile
from concourse import mybir
from concourse.bass_utils import run_bass_kernel_spmd

dt = mybir.dt
F16 = dt.float16
F32 = dt.float32
E4 = dt.float8e4
E5 = dt.float8e5
Alu = mybir.AluOpType
DR = mybir.MatmulPerfMode.DoubleRow

NCORES = 8
FULL = dict(T=16, IN=2048, H0=1024, H1=1024, OUT=512, BL=128)
NDIG = 6
EW = 4          # weight exponent for L1: W*2^EW in (-0.5, 0.5]
WFOLD = [4, 0, 4, 0, 4, 8]        # digit i stored as d_i * 2^-WFOLD[i]
KFOLD = [0, -8, -8, -16, -16, -16]  # spike plane scale exponents
# plane order in the sP tile: [s*2^t, s*2^(t-8), s*2^(t-16), s*2^(t-16)]
PLANE_OF_PAIR = [(0, 1), (1, 2), (2, 3)]  # pairs (d1,d2),(d3,d4),(d5,d6)

_BUILD_CACHE = {}


def build(T=16, IN=2048, H0=1024, H1=1024, OUT=512, BL=128):
    key = (T, IN, H0, H1, OUT, BL)
    if key in _BUILD_CACHE:
        return _BUILD_CACHE[key]
    KT0, KT1, KT2 = IN // 128, H0 // 128, H1 // 128
    MT0, MT1, MT2 = H0 // 128, H1 // 128, OUT // 128
    ZR = 512  # psum zero-region, fp32 elems (2KB)

    nc = bacc.Bacc("TRN2", target_bir_lowering=False, debug=False, num_devices=NCORES)

    # x hi/lo interleaved per 128-row group: rows (k,p,{hi,lo}) x BL cols —
    # keeps dram runs at 512B (no small-transfer DMA penalty), 1 DMA per tile
    xz_d = nc.dram_tensor("xz", [T, IN * 2, BL], F16, kind="ExternalInput")
    w0a_d = nc.dram_tensor("w0a", [IN, H0], F16, kind="ExternalInput")
    w0l_d = nc.dram_tensor("w0l", [IN, H0], F16, kind="ExternalInput")
    w1d_d = nc.dram_tensor("w1d", [H0, NDIG * H1], E4, kind="ExternalInput")
    woa_d = nc.dram_tensor("woa", [H1, OUT], F16, kind="ExternalInput")
    wol_d = nc.dram_tensor("wol", [H1, OUT], F16, kind="ExternalInput")
    br_d = {}
    for nm, h in (("br0", H0), ("br1", H1), ("br2", OUT)):
        br_d[nm] = nc.dram_tensor(nm, [2, h], F16, kind="ExternalInput")
    cst_d = {nm: nc.dram_tensor(nm, [2, 128], F16, kind="ExternalInput")
             for nm in ("cpos", "cneg0", "cneg1", "cneg2")}
    # out[p, (c,b)] = acc for out-neuron c*128+p, batch b (host de-permutes)
    out_d = nc.dram_tensor("out", [128, OUT // 128 * BL], F32,
                           kind="ExternalOutput")

    with tile.TileContext(nc) as tc:
        with tc.tile_pool(name="w", bufs=1) as wp, \
             tc.tile_pool(name="state", bufs=1) as sp, \
             tc.tile_pool(name="xs", bufs=2) as xp, \
             tc.tile_pool(name="spk", bufs=1) as kp, \
             tc.tile_pool(name="psum", bufs=1, space="PSUM") as pp:

            # ---- resident weights ----
            KH = KT0 // 2          # w0 split in 2 chunk-tiles for skew filler
            NX0 = KT0 // KH
            w_sb = {}
            for sfx in ("a", "l"):
                w_sb["w0" + sfx] = [
                    wp.tile([128, KH * H0], F16, tag=f"w0{sfx}{ci}",
                            name=f"w0{sfx}{ci}")
                    for ci in range(NX0)]
            w1d = wp.tile([128, KT1 * NDIG * H1], E4, tag="w1d", name="w1d")
            woa = wp.tile([128, KT2 * OUT], F16, tag="woa", name="woa")
            wol = wp.tile([128, KT2 * OUT], F16, tag="wol", name="wol")

            b_sb = {}
            for nm, h in (("br0", H0), ("br1", H1), ("br2", OUT)):
                b_sb[nm] = wp.tile([2, h], F16, tag=nm, name=nm)
            EL = {0: 0, 1: EW, 2: 0}   # per-layer PSUM weight exponent
            cpos = wp.tile([2, 128], F16, tag="cpos", name="cpos")
            cneg = {l: wp.tile([2, 128], F16, tag=f"cneg{l}", name=f"cneg{l}")
                    for l in (0, 1, 2)}

            # ---- states (free dim = (hidden-chunk, batch)) ----
            HS = {0: H0, 1: H1, 2: OUT}
            st = {}
            for l in (0, 1, 2):
                for nm in ("v0", "u0"):
                    st[(l, nm)] = sp.tile([128, HS[l]], F32, tag=f"{nm}{l}",
                                          name=f"{nm}{l}")
            vT = {0: sp.tile([128, H0], F32, tag="vT0", name="vT0"),
                  1: sp.tile([128, max(H1, OUT)], F32, tag="vT12", name="vT12")}
            vT[2] = vT[1]
            A_ = {0: sp.tile([128, H0], F32, tag="A0", name="A0"),
                  1: sp.tile([128, H1], F32, tag="A1", name="A1"),
                  2: sp.tile([128, OUT], F32, tag="A2", name="A2")}
            U_ = A_  # disjoint lifetimes: A dies at release, U born at post
            c021 = sp.tile([128, max(H0, H1)], F32, tag="c021")
            B2b = {0: sp.tile([128, H0], F32, tag="B2b0", name="B2b0"),
                   1: sp.tile([128, H1], F32, tag="B2b1", name="B2b1"),
                   2: sp.tile([128, OUT], F32, tag="B2b2", name="B2b2")}
            acc = sp.tile([128, OUT], F32, tag="acc", name="acc")
            C = {0: pp.tile([128, H0], F32, tag="C0", name="C0"),
                 1: pp.tile([128, H1], F32, tag="C1", name="C1"),
                 2: pp.tile([128, OUT], F32, tag="C2", name="C2")}
            C0b = pp.tile([128, H0], F32, tag="C0b", name="C0b")
            pB = pp.tile([128, 128], F32, tag="pB", name="pB")

            # ---- x loading ----
            x_pre = {}

            def load_x(t, cis=None):
                tiles = x_pre.setdefault(t, {})
                for ci in (cis if cis is not None else range(NX0)):
                    if ci in tiles:
                        continue
                    xz_t = xp.tile([128, KH * 2 * BL], F16, tag="xz",
                                   name=f"xz_t{t}_{ci}")
                    ks = ci * KH * 2 * 128
                    nc.sync.dma_start(
                        out=xz_t[:].rearrange("p (k two b) -> p k two b",
                                              two=2, b=BL),
                        in_=xz_d[t:t + 1, ks:ks + KH * 2 * 128].rearrange(
                            "o (k p two) b -> p (o k) two b", p=128, two=2))
                    tiles[ci] = xz_t

            def dma_w0(ci):
                for kk in range(KH):
                    k = ci * KH + kk
                    for sfx in ("a", "l"):
                        tl = w_sb["w0" + sfx][ci]
                        wd = w0a_d if sfx == "a" else w0l_d
                        nc.sync.dma_start(out=tl[:, kk * H0:(kk + 1) * H0],
                                          in_=wd[k * 128:(k + 1) * 128, :])

            def dma_w1d(ks):
                for k in ks:
                    nc.sync.dma_start(
                        out=w1d[:, k * NDIG * H1:(k + 1) * NDIG * H1],
                        in_=w1d_d[k * 128:(k + 1) * 128, :])

            def dma_wo():
                for k in range(KT2):
                    nc.sync.dma_start(out=woa[:, k * OUT:(k + 1) * OUT],
                                      in_=woa_d[k * 128:(k + 1) * 128, :])
                    nc.sync.dma_start(out=wol[:, k * OUT:(k + 1) * OUT],
                                      in_=wol_d[k * 128:(k + 1) * 128, :])

            # just-in-time DMA order (single serialized DMA resource)
            nc.sync.dma_start(out=cpos[:], in_=cst_d["cpos"][:])
            for l in (0, 1, 2):
                nc.sync.dma_start(out=cneg[l][:], in_=cst_d[f"cneg{l}"][:])
            load_x(0, cis=(0,))
            dma_w0(0)
            for nm in ("br0", "br1", "br2"):
                nc.sync.dma_start(out=b_sb[nm][:], in_=br_d[nm][:])
            load_x(0, cis=(1,))
            dma_w0(1)

            # init states + consts
            for l in (0, 1, 2):
                for nm in ("v0", "u0"):
                    nc.vector.memset(st[(l, nm)][:], 0.0)
            nc.vector.memset(c021[:], 0.021)
            nc.vector.memset(acc[:], 0.0)

            bias_of = {0: "br0", 1: "br1", 2: "br2"}

            def build_B2b():
                # B2b_l[p, (c,b)] = 2*b_l[c*128+p]  (PE outer product per chunk)
                for l in (0, 1, 2):
                    for m in range(HS[l] // 128):
                        nc.tensor.matmul(
                            pB[:], b_sb[bias_of[l]][:, m * 128:(m + 1) * 128],
                            cpos[:], start=True, stop=True,
                            skip_group_check=True)
                        nc.scalar.copy(B2b[l][:, m * 128:(m + 1) * 128], pB[:])

            def inject_bias(l):
                # add -b*2^EL[l] into each C[l] 128-chunk at t=0
                for m in range(HS[l] // 128):
                    nc.tensor.matmul(
                        C[l][:, m * 128:(m + 1) * 128],
                        b_sb[bias_of[l]][:, m * 128:(m + 1) * 128],
                        cneg[l][:], start=False, stop=False,
                        skip_group_check=True)

            # ---- L0 matmuls (fp16 3-term; stationary = w0 chunks) ----
            def emit_L0(t, cis):
                load_x(t, cis=cis)
                tiles = x_pre[t]
                for ci in cis:
                    xz_t = tiles.pop(ci)
                    if not tiles:
                        x_pre.pop(t, None)
                    wa = w_sb["w0a"][ci]
                    wl = w_sb["w0l"][ci]
                    for k in range(KH):
                        kg = ci * KH + k
                        ra = xz_t[:, (2 * k) * BL:(2 * k + 1) * BL]
                        rl = xz_t[:, (2 * k + 1) * BL:(2 * k + 2) * BL]
                        for m in range(MT0):
                            first = (t == 0 and kg == 0 and
                                     (m * 128) % ZR == 0)
                            last = (t == T - 1 and kg == KT0 - 1)
                            lwa = wa[:, k * H0 + m * 128:k * H0 + (m + 1) * 128]
                            lwl = wl[:, k * H0 + m * 128:k * H0 + (m + 1) * 128]
                            ps = C[0][:, m * 128:(m + 1) * 128]
                            psb = C0b[:, m * 128:(m + 1) * 128]
                            nc.tensor.matmul(ps, lwa, ra, start=first,
                                             stop=False, skip_group_check=True)
                            nc.tensor.matmul(psb, lwa, rl, start=first,
                                             stop=False, skip_group_check=True)
                            nc.tensor.matmul(psb, lwl, ra, start=False,
                                             stop=last, skip_group_check=True)
                    if t == 0 and ci == NX0 - 1:
                        inject_bias(0)

            # ---- L1: fp8 DoubleRow digit matmuls ----
            def emit_dr(t):
                sP = sP_cur[0]
                for k in range(KT1):
                    for m in range(MT1):
                        for pi, (pa, pb_) in enumerate(PLANE_OF_PAIR):
                            first = (t == 0 and k == 0 and pi == 0 and
                                     (m * 128) % ZR == 0)
                            last = (t == T - 1 and k == KT1 - 1 and pi == 2)
                            base = (k * NDIG + 2 * pi) * H1
                            lhs = w1d[:, base:base + 2 * H1].rearrange(
                                "p (two h) -> p two h", two=2)[
                                :, :, m * 128:(m + 1) * 128]
                            rhs = sP[:, pa * H0:(pa + 2) * H0].rearrange(
                                "p (two h) -> p two h", two=2)[
                                :, :, k * 128:(k + 1) * 128]
                            nc.tensor.matmul(
                                C[1][:, m * 128:(m + 1) * 128], lhs, rhs,
                                start=first, stop=last, perf_mode=DR,
                                skip_group_check=True)
                if t == 0:
                    inject_bias(1)

            # ---- fused LIF elementwise (layout-agnostic) ----
            def lif_pre(l, t):
                """Off-path: A = v0*v0 - u0 + B2b (ACT square + 2 DVE ops)."""
                h = HS[l]
                A = A_[l][:, :h]
                v0, u0 = st[(l, "v0")], st[(l, "u0")]
                nc.scalar.square(A, v0[:])
                nc.vector.tensor_tensor(out=A, in0=A, in1=u0[:],
                                        op=Alu.subtract)
                nc.vector.tensor_tensor(out=A, in0=A, in1=B2b[l][:], op=Alu.add)

            def lif_release(l, t):
                """DVE, reads PSUM: v = C*2^(-t-e) + A (+ C0b part for l=0)."""
                h = HS[l]
                v = vT[l][:, :h]
                nc.vector.scalar_tensor_tensor(
                    out=v, in0=C[l][:], scalar=float(2.0 ** (-t - EL[l])),
                    in1=A_[l][:, :h], op0=Alu.mult, op1=Alu.add)
                if l == 0:
                    nc.vector.scalar_tensor_tensor(
                        out=v, in0=C0b[:], scalar=float(2.0 ** -(t + 11)),
                        in1=v, op0=Alu.mult, op1=Alu.add)

            def lif_post(l, t, s_out, last):
                """Spike + state updates for step t+1."""
                h = HS[l]
                v = vT[l][:, :h]
                v0, u0 = st[(l, "v0")], st[(l, "u0")]
                s_scale = 1.0 if l == 2 else float(2.0 ** t)
                if l == 2:
                    nc.vector.scalar_tensor_tensor(
                        out=acc[:], in0=v, scalar=0.5, in1=acc[:],
                        op0=Alu.is_gt, op1=Alu.add)
                    if last:
                        return
                nc.vector.tensor_scalar(out=s_out, in0=v, scalar1=0.5,
                                        scalar2=s_scale, op0=Alu.is_gt,
                                        op1=Alu.mult)
                if last:
                    return
                U = U_[l][:, :h]
                nc.vector.scalar_tensor_tensor(
                    out=U, in0=v0[:], scalar=float(-0.172 / 1.529), in1=u0[:],
                    op0=Alu.mult, op1=Alu.add)
                nc.scalar.mul(U, U, 1.529)
                nc.vector.scalar_tensor_tensor(
                    out=u0[:], in0=s_out, scalar=float(0.132 / s_scale), in1=U,
                    op0=Alu.mult, op1=Alu.add)
                nc.scalar.copy(v0[:], v)
                nc.vector.copy_predicated(out=v0[:],
                                          mask=s_out.bitcast(dt.uint16),
                                          data=c021[:, :h])

            def make_planes(t, s0):
                """4 fp8e5 scaled copies of the L0 spikes (already s*2^t)."""
                sP = kp.tile([128, 4 * H0], E5, tag="sP0", name=f"sP0_t{t}")
                nc.scalar.copy(sP[:, 0:H0], s0[:])
                nc.vector.tensor_scalar(out=sP[:, H0:2 * H0], in0=s0[:],
                                        scalar1=float(2.0 ** -8), scalar2=None,
                                        op0=Alu.mult)
                nc.scalar.mul(sP[:, 2 * H0:3 * H0], s0[:], float(2.0 ** -16))
                nc.vector.tensor_scalar(out=sP[:, 3 * H0:4 * H0], in0=s0[:],
                                        scalar1=float(2.0 ** -16), scalar2=None,
                                        op0=Alu.mult)
                sP_cur[0] = sP

            sP_cur = [None]

            def emit_rest(t, filler=None):
                last = (t == T - 1)
                s0 = kp.tile([128, H0], F16, tag="s0", name=f"s0_t{t}")
                lif_post(0, t, s0[:], last)
                if not last:
                    lif_pre(0, t + 1)
                make_planes(t, s0)
                emit_dr(t)
                lif_release(1, t)
                if filler is not None:
                    filler()
                s1 = kp.tile([128, H1], F16, tag="s1", name=f"s1_t{t}")
                lif_post(1, t, s1[:], last)
                if not last:
                    lif_pre(1, t + 1)
                s1L = kp.tile([128, H1], F16, tag="s1L", name=f"s1L_t{t}")
                nc.vector.tensor_scalar(out=s1L[:], in0=s1[:],
                                        scalar1=float(2.0 ** -11), scalar2=None,
                                        op0=Alu.mult)
                # L2: fp16 hi/lo 2-term (stationary = wo chunks)
                for k in range(KT2):
                    ra = s1[:, k * 128:(k + 1) * 128]
                    rl = s1L[:, k * 128:(k + 1) * 128]
                    for m in range(MT2):
                        first = (t == 0 and k == 0 and (m * 128) % ZR == 0)
                        lastm = (t == T - 1 and k == KT2 - 1)
                        lwa = woa[:, k * OUT + m * 128:k * OUT + (m + 1) * 128]
                        lwl = wol[:, k * OUT + m * 128:k * OUT + (m + 1) * 128]
                        ps = C[2][:, m * 128:(m + 1) * 128]
                        nc.tensor.matmul(ps, lwa, ra, start=first, stop=False,
                                         skip_group_check=True)
                        nc.tensor.matmul(ps, lwl, rl, start=False, stop=lastm,
                                         skip_group_check=True)
                if t == 0:
                    inject_bias(2)
                lif_release(2, t)
                if last:
                    lif_post(2, t, None, last)
                else:
                    s2 = kp.tile([128, OUT], F16, tag="s2", name=f"s2_t{t}")
                    lif_post(2, t, s2[:], last)
                    lif_pre(2, t + 1)

            # ---- main loop: 1-step layer skew ----
            for t in range(T):
                if t >= 1:
                    lif_release(0, t - 1)   # frees C0/C0b for step t's matmuls
                emit_L0(t, cis=(0,))
                if t == 0:
                    dma_w1d(range(0, 4))
                    load_x(1, cis=(0,))
                    dma_w1d(range(4, KT1))
                    dma_wo()
                    load_x(1, cis=(1,))
                    build_B2b()
                    for l in (0, 1, 2):
                        lif_pre(l, 0)
                    emit_L0(0, cis=(1,))
                else:
                    emit_rest(t - 1, filler=lambda tt=t: emit_L0(tt, cis=(1,)))
                    if t + 1 < T:
                        load_x(t + 1)
            lif_release(0, T - 1)
            emit_rest(T - 1)

            nc.sync.dma_start(out=out_d[:], in_=acc[:])

    nc.compile()
    _BUILD_CACHE[key] = nc
    return nc


def _split_f16(a32, lo_scale=2048.0):
    hi = a32.astype(np.float16)
    lo = ((a32 - hi.astype(np.float32)) * np.float32(lo_scale)).astype(np.float16)
    return hi, lo


def _digit_planes(WT, ndig=NDIG, ew=EW):
    """WT [in,out] fp32 -> [in, ndig*out] fp8e4 digit planes (folded)."""
    r = WT.astype(np.float64) * (2.0 ** ew)
    assert np.max(np.abs(r)) <= 0.5, "weight exponent EW too small"
    planes = []
    for i in range(1, ndig + 1):
        di = np.rint(r * 16.0 ** i)
        di = np.clip(di, -4, 4) if i == ndig else np.clip(di, -8, 8)
        r = r - di * 16.0 ** -i
        planes.append(di * 2.0 ** -WFOLD[i - 1])
    out = np.concatenate(planes, axis=1).astype(ml_dtypes.float8_e4m3fn)
    assert np.all(out.astype(np.float64) == np.concatenate(planes, axis=1)), \
        "digit planes not exact in fp8e4"
    return out


def prep_inputs(in_pop_spikes, W0, b0, W1, b1, Wout, bout,
                T=16, BL=128, ncores=NCORES):
    x = np.ascontiguousarray(np.transpose(np.asarray(in_pop_spikes, np.float32),
                                          (2, 1, 0)))  # [T, IN, B]
    B = x.shape[2]
    IN = x.shape[1]
    scale = (2.0 ** np.arange(T, dtype=np.float32)).reshape(T, 1, 1)
    xh32 = x.astype(np.float16).astype(np.float32)
    xa = (xh32 * scale).astype(np.float16)
    xr = ((x - xh32) * (scale * np.float32(2048.0))).astype(np.float16)
    xz = np.stack([xa.reshape(T, IN // 128, 128, B),
                   xr.reshape(T, IN // 128, 128, B)], axis=3)
    xz = np.ascontiguousarray(xz.reshape(T, IN * 2, B))

    com = {}
    W0T = np.ascontiguousarray(np.asarray(W0, np.float32).T)
    com["w0a"], com["w0l"] = _split_f16(W0T)
    com["w1d"] = _digit_planes(np.ascontiguousarray(np.asarray(W1, np.float32).T))
    WoT = np.ascontiguousarray(np.asarray(Wout, np.float32).T)
    com["woa"], com["wol"] = _split_f16(WoT)
    for nm, b in (("br0", b0), ("br1", b1), ("br2", bout)):
        hi, lo = _split_f16(np.asarray(b, np.float32))
        com[nm] = np.stack([hi, lo])
    com["cpos"] = np.stack([np.full(128, 2.0, np.float16),
                            np.full(128, 2.0 / 2048.0, np.float16)])
    for l in (0, 1, 2):
        e = 2.0 ** EW if l == 1 else 1.0
        com[f"cneg{l}"] = np.stack([np.full(128, -e, np.float16),
                                    np.full(128, -e / 2048.0, np.float16)])

    in_maps = []
    for c in range(ncores):
        m = dict(com)
        m["xz"] = np.ascontiguousarray(xz[:, :, c * BL:(c + 1) * BL])
        in_maps.append(m)
    return in_maps


def kernel(in_pop_spikes, W0, b0, W1, b1, Wout, bout, batch_size, _trace=False):
    T = in_pop_spikes.shape[2]
    OUT, BL = Wout.shape[0], 128
    nc = build(**FULL)
    in_maps = prep_inputs(in_pop_spikes, W0, b0, W1, b1, Wout, bout, T=T)
    res = run_bass_kernel_spmd(nc, in_maps, core_ids=list(range(NCORES)),
                               trace=_trace)
    # device out[p, (c,b)] -> [b, c*128+p]
    outs = []
    for r in res.results:
        a = r["out"].reshape(128, OUT // 128, BL)
        outs.append(np.transpose(a, (2, 1, 0)).reshape(BL, OUT))
    out = (np.concatenate(outs, axis=0) / np.float32(T)).astype(np.float32)
    if _trace:
        kernel._last_results = res
    return out


# revision 62
# speedup vs baseline: 1.1431x; 1.0163x over previous
"""Trainium2 Bass kernel for nn_DynamicMLP (3-layer LIF spiking net, T=16).

Strategy (8 NeuronCores, data-parallel over batch):
  - Shard batch 1024 -> 8 x 128. Replicate weights. Zero cross-core comms.
  - TRANSPOSED layout: [hidden on partitions (128-chunks on free), batch on
    free dim]. Weights are the stationary matmul operand, activations the
    moving one; every layer's spikes come out exactly in the next layer's
    moving-operand layout, so there are NO DMA transposes anywhere.
  - LIF current state c lives in PSUM scaled by 2^t: C_t = sum 2^tau I_tau.
  - L0 (x @ W0): fp16 multi-term split, fp32-exact to ~2^-22 (the network is
    chaotic: >=20 significant bits needed on BOTH operands; measured):
      w0h.T@xh -> C0;  w0h.T@(2^11*xl) and (w0l*2^11).T@xh -> C0b,
    folded at 2^-(t+11) on release.
  - L1 (spikes @ W1): W1 decomposed into 6 signed radix-16 digits stored as
    exact fp8e4 planes (stationary); spikes as fp8e5 planes at 3 scales
    {2^t, 2^(t-8), 2^(t-16)} (moving). Matmuls run as fp8 DoubleRow pairs
    (2 digit products per instr at 0.5 cycles/row): 25% fewer PE cycles than
    the fp16 hi/lo split, ~21.5-bit effective weights (verified on device).
  - L2: fp16 hi/lo 2-term (digit planes for it don't fit SBUF).
  - Biases: one -b*2^e matmul injected into each C group at t=0; the +2b
    constant enters through the fused v-update (c_t = C*2^-t - b*2^-t + 2b).
    No per-step bias matmuls.
  - Fused LIF elementwise (v_t = v0^2 - u0 + c) split across DVE + ACT; the
    per-layer A = v0^2 - u0 + B2b is precomputed one step early off the
    critical path. Output accumulation (acc += v>vth) on DVE, not the PE.
"""
import sys

sys.path.insert(0, "/opt/trn_rl_repo")

import numpy as np
import ml_dtypes

import concourse.bacc as bacc
import concourse.tile as tile
from concourse import mybir
from concourse.bass_utils import run_bass_kernel_spmd

dt = mybir.dt
F16 = dt.float16
F32 = dt.float32
E4 = dt.float8e4
E5 = dt.float8e5
Alu = mybir.AluOpType
DR = mybir.MatmulPerfMode.DoubleRow

NCORES = 8
FULL = dict(T=16, IN=2048, H0=1024, H1=1024, OUT=512, BL=128)
NDIG = 6
EW = 4          # weight exponent for L1: W*2^EW in (-0.5, 0.5]
WFOLD = [4, 0, 4, 0, 4, 8]        # digit i stored as d_i * 2^-WFOLD[i]
KFOLD = [0, -8, -8, -16, -16, -16]  # spike plane scale exponents
# plane order in the sP tile: [s*2^t, s*2^(t-8), s*2^(t-16), s*2^(t-16)]
PLANE_OF_PAIR = [(0, 1), (1, 2), (2, 3)]  # pairs (d1,d2),(d3,d4),(d5,d6)

_BUILD_CACHE = {}


def build(T=16, IN=2048, H0=1024, H1=1024, OUT=512, BL=128):
    key = (T, IN, H0, H1, OUT, BL)
    if key in _BUILD_CACHE:
        return _BUILD_CACHE[key]
    KT0, KT1, KT2 = IN // 128, H0 // 128, H1 // 128
    MT0, MT1, MT2 = H0 // 128, H1 // 128, OUT // 128
    ZR = 512  # psum zero-region, fp32 elems (2KB)

    nc = bacc.Bacc("TRN2", target_bir_lowering=False, debug=False, num_devices=NCORES)

    # x hi/lo interleaved per 128-row group: rows (k,p,{hi,lo}) x BL cols —
    # keeps dram runs at 512B (no small-transfer DMA penalty), 1 DMA per tile
    xz_d = nc.dram_tensor("xz", [T, IN * 2, BL], F16, kind="ExternalInput")
    w0a_d = nc.dram_tensor("w0a", [IN, H0], F16, kind="ExternalInput")
    w0l_d = nc.dram_tensor("w0l", [IN, H0], F16, kind="ExternalInput")
    w1d_d = nc.dram_tensor("w1d", [H0, NDIG * H1], E4, kind="ExternalInput")
    woa_d = nc.dram_tensor("woa", [H1, OUT], F16, kind="ExternalInput")
    wol_d = nc.dram_tensor("wol", [H1, OUT], F16, kind="ExternalInput")
    br_d = {}
    for nm, h in (("br0", H0), ("br1", H1), ("br2", OUT)):
        br_d[nm] = nc.dram_tensor(nm, [2, h], F16, kind="ExternalInput")
    cst_d = {nm: nc.dram_tensor(nm, [2, 128], F16, kind="ExternalInput")
             for nm in ("cpos", "cneg0", "cneg1", "cneg2")}
    # out[p, (c,b)] = acc for out-neuron c*128+p, batch b (host de-permutes)
    out_d = nc.dram_tensor("out", [128, OUT // 128 * BL], F32,
                           kind="ExternalOutput")

    with tile.TileContext(nc) as tc:
        with tc.tile_pool(name="w", bufs=1) as wp, \
             tc.tile_pool(name="state", bufs=1) as sp, \
             tc.tile_pool(name="xs", bufs=2) as xp, \
             tc.tile_pool(name="spk", bufs=1) as kp, \
             tc.tile_pool(name="psum", bufs=1, space="PSUM") as pp:

            # ---- resident weights ----
            KH = KT0 // 2          # w0 split in 2 chunk-tiles for skew filler
            NX0 = KT0 // KH
            w_sb = {}
            for sfx in ("a", "l"):
                w_sb["w0" + sfx] = [
                    wp.tile([128, KH * H0], F16, tag=f"w0{sfx}{ci}",
                            name=f"w0{sfx}{ci}")
                    for ci in range(NX0)]
            w1d = wp.tile([128, KT1 * NDIG * H1], E4, tag="w1d", name="w1d")
            woa = wp.tile([128, KT2 * OUT], F16, tag="woa", name="woa")
            wol = wp.tile([128, KT2 * OUT], F16, tag="wol", name="wol")

            b_sb = {}
            for nm, h in (("br0", H0), ("br1", H1), ("br2", OUT)):
                b_sb[nm] = wp.tile([2, h], F16, tag=nm, name=nm)
            EL = {0: 0, 1: EW, 2: 0}   # per-layer PSUM weight exponent
            cpos = wp.tile([2, 128], F16, tag="cpos", name="cpos")
            cneg = {l: wp.tile([2, 128], F16, tag=f"cneg{l}", name=f"cneg{l}")
                    for l in (0, 1, 2)}

            # ---- states (free dim = (hidden-chunk, batch)) ----
            HS = {0: H0, 1: H1, 2: OUT}
            st = {}
            for l in (0, 1, 2):
                for nm in ("v0", "u0"):
                    st[(l, nm)] = sp.tile([128, HS[l]], F32, tag=f"{nm}{l}",
                                          name=f"{nm}{l}")
            vT = {0: sp.tile([128, H0], F32, tag="vT0", name="vT0"),
                  1: sp.tile([128, max(H1, OUT)], F32, tag="vT12", name="vT12")}
            vT[2] = vT[1]
            A_ = {0: sp.tile([128, H0], F32, tag="A0", name="A0"),
                  1: sp.tile([128, H1], F32, tag="A1", name="A1"),
                  2: sp.tile([128, OUT], F32, tag="A2", name="A2")}
            U_ = A_  # disjoint lifetimes: A dies at release, U born at post
            c021 = sp.tile([128, max(H0, H1)], F32, tag="c021")
            B2b = {0: sp.tile([128, H0], F32, tag="B2b0", name="B2b0"),
                   1: sp.tile([128, H1], F32, tag="B2b1", name="B2b1"),
                   2: sp.tile([128, OUT], F32, tag="B2b2", name="B2b2")}
            acc = sp.tile([128, OUT], F32, tag="acc", name="acc")
            C = {0: pp.tile([128, H0], F32, tag="C0", name="C0"),
                 1: pp.tile([128, H1], F32, tag="C1", name="C1"),
                 2: pp.tile([128, OUT], F32, tag="C2", name="C2")}
            C0b = pp.tile([128, H0], F32, tag="C0b", name="C0b")
            pB = pp.tile([128, 512], F32, tag="pB", name="pB")

            # ---- x loading ----
            x_pre = {}

            def load_x(t, cis=None):
                tiles = x_pre.setdefault(t, {})
                for ci in (cis if cis is not None else range(NX0)):
                    if ci in tiles:
                        continue
                    xz_t = xp.tile([128, KH * 2 * BL], F16, tag="xz",
                                   name=f"xz_t{t}_{ci}")
                    ks = ci * KH * 2 * 128
                    nc.sync.dma_start(
                        out=xz_t[:].rearrange("p (k two b) -> p k two b",
                                              two=2, b=BL),
                        in_=xz_d[t:t + 1, ks:ks + KH * 2 * 128].rearrange(
                            "o (k p two) b -> p (o k) two b", p=128, two=2))
                    tiles[ci] = xz_t

            def dma_w0(ci):
                for kk in range(KH):
                    k = ci * KH + kk
                    for sfx in ("a", "l"):
                        tl = w_sb["w0" + sfx][ci]
                        wd = w0a_d if sfx == "a" else w0l_d
                        nc.sync.dma_start(out=tl[:, kk * H0:(kk + 1) * H0],
                                          in_=wd[k * 128:(k + 1) * 128, :])

            def dma_w1d(ks):
                for k in ks:
                    nc.sync.dma_start(
                        out=w1d[:, k * NDIG * H1:(k + 1) * NDIG * H1],
                        in_=w1d_d[k * 128:(k + 1) * 128, :])

            def dma_wo():
                for k in range(KT2):
                    nc.sync.dma_start(out=woa[:, k * OUT:(k + 1) * OUT],
                                      in_=woa_d[k * 128:(k + 1) * 128, :])
                    nc.sync.dma_start(out=wol[:, k * OUT:(k + 1) * OUT],
                                      in_=wol_d[k * 128:(k + 1) * 128, :])

            # just-in-time DMA order (single serialized DMA resource)
            nc.sync.dma_start(out=cpos[:], in_=cst_d["cpos"][:])
            for l in (0, 1, 2):
                nc.sync.dma_start(out=cneg[l][:], in_=cst_d[f"cneg{l}"][:])
            load_x(0, cis=(0,))
            dma_w0(0)
            for nm in ("br0", "br1", "br2"):
                nc.sync.dma_start(out=b_sb[nm][:], in_=br_d[nm][:])
            dma_w1d(range(0, 2))
            load_x(0, cis=(1,))
            dma_w0(1)

            # init states + consts
            for l in (0, 1, 2):
                for nm in ("v0", "u0"):
                    nc.vector.memset(st[(l, nm)][:], 0.0)
            nc.vector.memset(c021[:], 0.021)
            nc.vector.memset(acc[:], 0.0)

            bias_of = {0: "br0", 1: "br1", 2: "br2"}

            def build_B2b():
                # B2b_l[p, (c,b)] = 2*b_l[c*128+p]  (PE outer product per
                # chunk, staggered over 4 bank slots to avoid WAR ping-pong)
                i = 0
                for l in (0, 1, 2):
                    for m in range(HS[l] // 128):
                        sl = 0
                        i += 1
                        nc.tensor.matmul(
                            pB[:, sl:sl + 128],
                            b_sb[bias_of[l]][:, m * 128:(m + 1) * 128],
                            cpos[:], start=True, stop=True,
                            skip_group_check=True)
                        nc.scalar.copy(B2b[l][:, m * 128:(m + 1) * 128],
                                       pB[:, sl:sl + 128])

            def inject_bias(l):
                # add -b*2^EL[l] into each C[l] 128-chunk at t=0
                for m in range(HS[l] // 128):
                    nc.tensor.matmul(
                        C[l][:, m * 128:(m + 1) * 128],
                        b_sb[bias_of[l]][:, m * 128:(m + 1) * 128],
                        cneg[l][:], start=False, stop=False,
                        skip_group_check=True)

            # ---- L0 matmuls (fp16 3-term; stationary = w0 chunks) ----
            def emit_L0(t, cis):
                load_x(t, cis=cis)
                tiles = x_pre[t]
                for ci in cis:
                    xz_t = tiles.pop(ci)
                    if not tiles:
                        x_pre.pop(t, None)
                    wa = w_sb["w0a"][ci]
                    wl = w_sb["w0l"][ci]
                    if True:
                        for k in range(KH):
                            kg = ci * KH + k
                            ra = xz_t[:, (2 * k) * BL:(2 * k + 1) * BL]
                            rl = xz_t[:, (2 * k + 1) * BL:(2 * k + 2) * BL]
                            for m in range(MT0):
                                first = (t == 0 and kg == 0 and
                                         (m * 128) % ZR == 0)
                                last = (t == T - 1 and kg == KT0 - 1)
                                lwa = wa[:, k * H0 + m * 128:
                                         k * H0 + (m + 1) * 128]
                                lwl = wl[:, k * H0 + m * 128:
                                         k * H0 + (m + 1) * 128]
                                ps = C[0][:, m * 128:(m + 1) * 128]
                                psb = C0b[:, m * 128:(m + 1) * 128]
                                nc.tensor.matmul(ps, lwa, ra, start=first,
                                                 stop=False,
                                                 skip_group_check=True)
                                nc.tensor.matmul(psb, lwa, rl, start=first,
                                                 stop=False,
                                                 skip_group_check=True)
                                nc.tensor.matmul(psb, lwl, ra, start=False,
                                                 stop=last,
                                                 skip_group_check=True)
                    if t == 0 and ci == NX0 - 1:
                        inject_bias(0)

            # ---- L1: fp8 DoubleRow digit matmuls ----
            def emit_dr(t):
                sP = sP_cur[0]
                for k in range(KT1):
                    for m in range(MT1):
                        for pi, (pa, pb_) in enumerate(PLANE_OF_PAIR):
                            first = (t == 0 and k == 0 and pi == 0 and
                                     (m * 128) % ZR == 0)
                            last = (t == T - 1 and k == KT1 - 1 and pi == 2)
                            base = (k * NDIG + 2 * pi) * H1
                            lhs = w1d[:, base:base + 2 * H1].rearrange(
                                "p (two h) -> p two h", two=2)[
                                :, :, m * 128:(m + 1) * 128]
                            rhs = sP[:, pa * H0:(pa + 2) * H0].rearrange(
                                "p (two h) -> p two h", two=2)[
                                :, :, k * 128:(k + 1) * 128]
                            nc.tensor.matmul(
                                C[1][:, m * 128:(m + 1) * 128], lhs, rhs,
                                start=first, stop=last, perf_mode=DR,
                                skip_group_check=True)
                if t == 0:
                    inject_bias(1)

            # ---- fused LIF elementwise (layout-agnostic) ----
            def lif_pre(l, t):
                """Off-path: A = v0*v0 - u0 + B2b (ACT square + 2 DVE ops)."""
                h = HS[l]
                A = A_[l][:, :h]
                v0, u0 = st[(l, "v0")], st[(l, "u0")]
                nc.scalar.square(A, v0[:])
                nc.vector.tensor_tensor(out=A, in0=A, in1=u0[:],
                                        op=Alu.subtract)
                nc.vector.tensor_tensor(out=A, in0=A, in1=B2b[l][:], op=Alu.add)

            def lif_release(l, t):
                """DVE, reads PSUM: v = C*2^(-t-e) + A (+ C0b part for l=0).
                Emitted per bank-half so downstream consumers start early."""
                h = HS[l]
                hh = h
                for off in range(0, h, hh):
                    v = vT[l][:, off:off + hh]
                    nc.vector.scalar_tensor_tensor(
                        out=v, in0=C[l][:, off:off + hh],
                        scalar=float(2.0 ** (-t - EL[l])),
                        in1=A_[l][:, off:off + hh], op0=Alu.mult, op1=Alu.add)
                    if l == 0:
                        nc.vector.scalar_tensor_tensor(
                            out=v, in0=C0b[:, off:off + hh],
                            scalar=float(2.0 ** -(t + 11)),
                            in1=v, op0=Alu.mult, op1=Alu.add)

            def lif_spike(l, t, s_out, off, hh):
                """Spike threshold for one half (chain-critical)."""
                s_scale = 1.0 if l == 2 else float(2.0 ** t)
                nc.vector.tensor_scalar(
                    out=s_out[:, off:off + hh], in0=vT[l][:, off:off + hh],
                    scalar1=0.5, scalar2=s_scale, op0=Alu.is_gt, op1=Alu.mult)

            def lif_states(l, t, s_out, last):
                """State updates for step t+1 (off critical path)."""
                h = HS[l]
                v = vT[l][:, :h]
                v0, u0 = st[(l, "v0")], st[(l, "u0")]
                s_scale = 1.0 if l == 2 else float(2.0 ** t)
                if last:
                    return
                U = U_[l][:, :h]
                nc.vector.scalar_tensor_tensor(
                    out=U, in0=v0[:], scalar=float(-0.172 / 1.529), in1=u0[:],
                    op0=Alu.mult, op1=Alu.add)
                nc.scalar.mul(U, U, 1.529)
                nc.vector.scalar_tensor_tensor(
                    out=u0[:], in0=s_out[:], scalar=float(0.132 / s_scale),
                    in1=U, op0=Alu.mult, op1=Alu.add)
                nc.scalar.copy(v0[:], v)
                nc.vector.copy_predicated(out=v0[:],
                                          mask=s_out[:].bitcast(dt.uint16),
                                          data=c021[:, :h])

            def make_planes_half(s0, sP, off, hh):
                """4 fp8e5 scaled copies of one half of the L0 spikes."""
                nc.scalar.copy(sP[:, off:off + hh], s0[:, off:off + hh])
                nc.vector.tensor_scalar(
                    out=sP[:, H0 + off:H0 + off + hh], in0=s0[:, off:off + hh],
                    scalar1=float(2.0 ** -8), scalar2=None, op0=Alu.mult)
                nc.scalar.mul(sP[:, 2 * H0 + off:2 * H0 + off + hh],
                              s0[:, off:off + hh], float(2.0 ** -16))
                nc.scalar.mul(sP[:, 3 * H0 + off:3 * H0 + off + hh],
                              s0[:, off:off + hh], float(2.0 ** -16))

            sP_cur = [None]

            def emit_rest(t, filler=None):
                last = (t == T - 1)
                s0 = kp.tile([128, H0], F16, tag="s0", name=f"s0_t{t}")
                sP = kp.tile([128, 4 * H0], E5, tag="sP0", name=f"sP0_t{t}")
                sP_cur[0] = sP
                lif_spike(0, t, s0, 0, H0)
                lif_states(0, t, s0, last)
                if not last:
                    lif_pre(0, t + 1)
                make_planes_half(s0, sP, 0, H0)
                emit_dr(t)
                lif_release(1, t)
                if filler is not None:
                    filler()
                s1 = kp.tile([128, H1], F16, tag="s1", name=f"s1_t{t}")
                s1L = kp.tile([128, H1], F16, tag="s1L", name=f"s1L_t{t}")
                lif_spike(1, t, s1, 0, H1)
                lif_states(1, t, s1, last)
                if not last:
                    lif_pre(1, t + 1)
                nc.vector.tensor_scalar(out=s1L[:], in0=s1[:],
                                        scalar1=float(2.0 ** -11),
                                        scalar2=None, op0=Alu.mult)
                # L2: fp16 hi/lo 2-term (stationary = wo chunks)
                for k in range(KT2):
                    ra = s1[:, k * 128:(k + 1) * 128]
                    rl = s1L[:, k * 128:(k + 1) * 128]
                    for m in range(MT2):
                        first = (t == 0 and k == 0 and (m * 128) % ZR == 0)
                        lastm = (t == T - 1 and k == KT2 - 1)
                        lwa = woa[:, k * OUT + m * 128:k * OUT + (m + 1) * 128]
                        lwl = wol[:, k * OUT + m * 128:k * OUT + (m + 1) * 128]
                        ps = C[2][:, m * 128:(m + 1) * 128]
                        nc.tensor.matmul(ps, lwa, ra, start=first, stop=False,
                                         skip_group_check=True)
                        nc.tensor.matmul(ps, lwl, rl, start=False, stop=lastm,
                                         skip_group_check=True)
                if t == 0:
                    inject_bias(2)
                lif_release(2, t)
                nc.vector.scalar_tensor_tensor(
                    out=acc[:], in0=vT[2][:, :OUT], scalar=0.5, in1=acc[:],
                    op0=Alu.is_gt, op1=Alu.add)
                if not last:
                    s2 = kp.tile([128, OUT], F16, tag="s2", name=f"s2_t{t}")
                    lif_spike(2, t, s2, 0, OUT)
                    lif_states(2, t, s2, last)
                    lif_pre(2, t + 1)

            # ---- main loop: 1-step layer skew ----
            for t in range(T):
                if t >= 1:
                    lif_release(0, t - 1)   # frees C0/C0b for step t's matmuls
                emit_L0(t, cis=(0,))
                if t == 0:
                    load_x(1, cis=(0,))
                    dma_w1d(range(2, KT1))
                    dma_wo()
                    load_x(1, cis=(1,))
                    build_B2b()
                    for l in (0, 1, 2):
                        lif_pre(l, 0)
                    emit_L0(0, cis=(1,))
                else:
                    emit_rest(t - 1, filler=lambda tt=t: emit_L0(tt, cis=(1,)))
                    if t + 1 < T:
                        load_x(t + 1)
            lif_release(0, T - 1)
            emit_rest(T - 1)

            nc.sync.dma_start(out=out_d[:], in_=acc[:])

    nc.compile()
    _BUILD_CACHE[key] = nc
    return nc


def _split_f16(a32, lo_scale=2048.0):
    hi = a32.astype(np.float16)
    lo = ((a32 - hi.astype(np.float32)) * np.float32(lo_scale)).astype(np.float16)
    return hi, lo


def _digit_planes(WT, ndig=NDIG, ew=EW):
    """WT [in,out] fp32 -> [in, ndig*out] fp8e4 digit planes (folded)."""
    r = WT.astype(np.float64) * (2.0 ** ew)
    assert np.max(np.abs(r)) <= 0.5, "weight exponent EW too small"
    planes = []
    for i in range(1, ndig + 1):
        di = np.rint(r * 16.0 ** i)
        di = np.clip(di, -4, 4) if i == ndig else np.clip(di, -8, 8)
        r = r - di * 16.0 ** -i
        planes.append(di * 2.0 ** -WFOLD[i - 1])
    out = np.concatenate(planes, axis=1).astype(ml_dtypes.float8_e4m3fn)
    assert np.all(out.astype(np.float64) == np.concatenate(planes, axis=1)), \
        "digit planes not exact in fp8e4"
    return out


def prep_inputs(in_pop_spikes, W0, b0, W1, b1, Wout, bout,
                T=16, BL=128, ncores=NCORES):
    x = np.ascontiguousarray(np.transpose(np.asarray(in_pop_spikes, np.float32),
                                          (2, 1, 0)))  # [T, IN, B]
    B = x.shape[2]
    IN = x.shape[1]
    scale = (2.0 ** np.arange(T, dtype=np.float32)).reshape(T, 1, 1)
    xh32 = x.astype(np.float16).astype(np.float32)
    xa = (xh32 * scale).astype(np.float16)
    xr = ((x - xh32) * (scale * np.float32(2048.0))).astype(np.float16)
    xz = np.stack([xa.reshape(T, IN // 128, 128, B),
                   xr.reshape(T, IN // 128, 128, B)], axis=3)
    xz = np.ascontiguousarray(xz.reshape(T, IN * 2, B))

    com = {}
    W0T = np.ascontiguousarray(np.asarray(W0, np.float32).T)
    com["w0a"], com["w0l"] = _split_f16(W0T)
    com["w1d"] = _digit_planes(np.ascontiguousarray(np.asarray(W1, np.float32).T))
    WoT = np.ascontiguousarray(np.asarray(Wout, np.float32).T)
    com["woa"], com["wol"] = _split_f16(WoT)
    for nm, b in (("br0", b0), ("br1", b1), ("br2", bout)):
        hi, lo = _split_f16(np.asarray(b, np.float32))
        com[nm] = np.stack([hi, lo])
    com["cpos"] = np.stack([np.full(128, 2.0, np.float16),
                            np.full(128, 2.0 / 2048.0, np.float16)])
    for l in (0, 1, 2):
        e = 2.0 ** EW if l == 1 else 1.0
        com[f"cneg{l}"] = np.stack([np.full(128, -e, np.float16),
                                    np.full(128, -e / 2048.0, np.float16)])

    in_maps = []
    for c in range(ncores):
        m = dict(com)
        m["xz"] = np.ascontiguousarray(xz[:, :, c * BL:(c + 1) * BL])
        in_maps.append(m)
    return in_maps


def kernel(in_pop_spikes, W0, b0, W1, b1, Wout, bout, batch_size, _trace=False):
    T = in_pop_spikes.shape[2]
    OUT, BL = Wout.shape[0], 128
    nc = build(**FULL)
    in_maps = prep_inputs(in_pop_spikes, W0, b0, W1, b1, Wout, bout, T=T)
    res = run_bass_kernel_spmd(nc, in_maps, core_ids=list(range(NCORES)),
                               trace=_trace)
    # device out[p, (c,b)] -> [b, c*128+p]
    outs = []
    for r in res.results:
        a = r["out"].reshape(128, OUT // 128, BL)
        outs.append(np.transpose(a, (2, 1, 0)).reshape(BL, OUT))
    out = (np.concatenate(outs, axis=0) / np.float32(T)).astype(np.float32)
    if _trace:
        kernel._last_results = res
    return out


# revision 75
# speedup vs baseline: 1.1761x; 1.0289x over previous
"""Trainium2 Bass kernel for nn_DynamicMLP (3-layer LIF spiking net, T=16).

Strategy (8 NeuronCores, data-parallel over batch):
  - Shard batch 1024 -> 8 x 128. Replicate weights. Zero cross-core comms.
  - TRANSPOSED layout: [hidden on partitions (128-chunks on free), batch on
    free dim]. Weights are the stationary matmul operand, activations the
    moving one; every layer's spikes come out exactly in the next layer's
    moving-operand layout, so there are NO DMA transposes anywhere.
  - LIF current state c lives in PSUM scaled by 2^t: C_t = sum 2^tau I_tau.
  - L0 (x @ W0): fp16 multi-term split, fp32-exact to ~2^-22 (the network is
    chaotic: >=20 significant bits needed on BOTH operands; measured):
      w0h.T@xh -> C0;  w0h.T@(2^11*xl) and (w0l*2^11).T@xh -> C0b,
    folded at 2^-(t+11) on release.
  - L1 (spikes @ W1): W1 decomposed into 6 signed radix-16 digits stored as
    exact fp8e4 planes (stationary); spikes as fp8e5 planes at 3 scales
    {2^t, 2^(t-8), 2^(t-16)} (moving). Matmuls run as fp8 DoubleRow pairs
    (2 digit products per instr at 0.5 cycles/row): 25% fewer PE cycles than
    the fp16 hi/lo split, ~21.5-bit effective weights (verified on device).
  - L2: fp16 hi/lo 2-term (digit planes for it don't fit SBUF).
  - Biases: one -b*2^e matmul injected into each C group at t=0; the +2b
    constant enters through the fused v-update (c_t = C*2^-t - b*2^-t + 2b).
    No per-step bias matmuls.
  - Fused LIF elementwise (v_t = v0^2 - u0 + c) split across DVE + ACT; the
    per-layer A = v0^2 - u0 + B2b is precomputed one step early off the
    critical path. Output accumulation (acc += v>vth) on DVE, not the PE.
"""
import sys

sys.path.insert(0, "/opt/trn_rl_repo")

import numpy as np
import ml_dtypes

import concourse.bacc as bacc
import concourse.tile as tile
from concourse import mybir
from concourse.bass_utils import run_bass_kernel_spmd

dt = mybir.dt
F16 = dt.float16
F32 = dt.float32
E4 = dt.float8e4
E5 = dt.float8e5
Alu = mybir.AluOpType
DR = mybir.MatmulPerfMode.DoubleRow

NCORES = 8
FULL = dict(T=16, IN=2048, H0=1024, H1=1024, OUT=512, BL=128)
NDIG = 6
EW = 4          # weight exponent for L1: W*2^EW in (-0.5, 0.5]
WFOLD = [4, 0, 4, 0, 4, 8]        # digit i stored as d_i * 2^-WFOLD[i]
KFOLD = [0, -8, -8, -16, -16, -16]  # spike plane scale exponents
# plane order in the sP tile: [s*2^t, s*2^(t-8), s*2^(t-16), s*2^(t-16)]
PLANE_OF_PAIR = [(0, 1), (1, 2), (2, 3)]  # pairs (d1,d2),(d3,d4),(d5,d6)

_BUILD_CACHE = {}


def build(T=16, IN=2048, H0=1024, H1=1024, OUT=512, BL=128):
    key = (T, IN, H0, H1, OUT, BL)
    if key in _BUILD_CACHE:
        return _BUILD_CACHE[key]
    KT0, KT1, KT2 = IN // 128, H0 // 128, H1 // 128
    MT0, MT1, MT2 = H0 // 128, H1 // 128, OUT // 128
    ZR = 512  # psum zero-region, fp32 elems (2KB)

    nc = bacc.Bacc("TRN2", target_bir_lowering=False, debug=False, num_devices=NCORES)

    # x hi/lo interleaved per 128-row group: rows (k,p,{hi,lo}) x BL cols —
    # keeps dram runs at 512B (no small-transfer DMA penalty), 1 DMA per tile
    xz_d = nc.dram_tensor("xz", [T, IN * 2, BL], F16, kind="ExternalInput")
    w0a_d = nc.dram_tensor("w0a", [IN, H0], F16, kind="ExternalInput")
    w0l_d = nc.dram_tensor("w0l", [IN, H0], F16, kind="ExternalInput")
    w1d_d = nc.dram_tensor("w1d", [H0, NDIG * H1], E4, kind="ExternalInput")
    woa_d = nc.dram_tensor("woa", [H1, OUT], F16, kind="ExternalInput")
    wol_d = nc.dram_tensor("wol", [H1, OUT], F16, kind="ExternalInput")
    br_d = {}
    for nm, h in (("br0", H0), ("br1", H1), ("br2", OUT)):
        br_d[nm] = nc.dram_tensor(nm, [2, h], F16, kind="ExternalInput")
    cst_d = {nm: nc.dram_tensor(nm, [2, 128], F16, kind="ExternalInput")
             for nm in ("cpos", "cneg0", "cneg1", "cneg2")}
    # out[p, (c,b)] = acc for out-neuron c*128+p, batch b (host de-permutes)
    out_d = nc.dram_tensor("out", [128, OUT // 128 * BL], F32,
                           kind="ExternalOutput")

    with tile.TileContext(nc) as tc:
        with tc.tile_pool(name="w", bufs=1) as wp, \
             tc.tile_pool(name="state", bufs=1) as sp, \
             tc.tile_pool(name="xs", bufs=2) as xp, \
             tc.tile_pool(name="spk", bufs=1) as kp, \
             tc.tile_pool(name="psum", bufs=1, space="PSUM") as pp:

            # ---- resident weights ----
            KH = KT0 // 2          # w0 split in 2 chunk-tiles for skew filler
            NX0 = KT0 // KH
            w_sb = {}
            for sfx in ("a", "l"):
                w_sb["w0" + sfx] = [
                    wp.tile([128, KH * H0], F16, tag=f"w0{sfx}{ci}",
                            name=f"w0{sfx}{ci}")
                    for ci in range(NX0)]
            w1d = wp.tile([128, KT1 * NDIG * H1], E4, tag="w1d", name="w1d")
            woa = wp.tile([128, KT2 * OUT], F16, tag="woa", name="woa")
            wol = wp.tile([128, KT2 * OUT], F16, tag="wol", name="wol")

            b_sb = {}
            for nm, h in (("br0", H0), ("br1", H1), ("br2", OUT)):
                b_sb[nm] = wp.tile([2, h], F16, tag=nm, name=nm)
            EL = {0: 0, 1: EW, 2: 0}   # per-layer PSUM weight exponent
            cpos = wp.tile([2, 128], F16, tag="cpos", name="cpos")
            cneg = {l: wp.tile([2, 128], F16, tag=f"cneg{l}", name=f"cneg{l}")
                    for l in (0, 1, 2)}

            # ---- states (free dim = (hidden-chunk, batch)) ----
            HS = {0: H0, 1: H1, 2: OUT}
            st = {}
            for l in (0, 1, 2):
                for nm in ("v0", "u0"):
                    st[(l, nm)] = sp.tile([128, HS[l]], F32, tag=f"{nm}{l}",
                                          name=f"{nm}{l}")
            vT = {0: sp.tile([128, H0], F32, tag="vT0", name="vT0"),
                  1: sp.tile([128, max(H1, OUT)], F32, tag="vT12", name="vT12")}
            vT[2] = vT[1]
            A_ = {0: sp.tile([128, H0], F32, tag="A0", name="A0"),
                  1: sp.tile([128, H1], F32, tag="A1", name="A1"),
                  2: sp.tile([128, OUT], F32, tag="A2", name="A2")}
            U_ = A_  # disjoint lifetimes: A dies at release, U born at post
            c021 = sp.tile([128, max(H0, H1)], F32, tag="c021")
            B2b = {0: sp.tile([128, H0], F32, tag="B2b0", name="B2b0"),
                   1: sp.tile([128, H1], F32, tag="B2b1", name="B2b1"),
                   2: sp.tile([128, OUT], F32, tag="B2b2", name="B2b2")}
            acc = sp.tile([128, OUT], F32, tag="acc", name="acc")
            C = {0: pp.tile([128, H0], F32, tag="C0", name="C0"),
                 1: pp.tile([128, H1], F32, tag="C1", name="C1"),
                 2: pp.tile([128, OUT], F32, tag="C2", name="C2")}
            C0b = pp.tile([128, H0], F32, tag="C0b", name="C0b")
            pB = pp.tile([128, 512], F32, tag="pB", name="pB")

            # ---- x loading ----
            x_pre = {}

            def load_x(t, cis=None):
                tiles = x_pre.setdefault(t, {})
                for ci in (cis if cis is not None else range(NX0)):
                    if ci in tiles:
                        continue
                    xz_t = xp.tile([128, KH * 2 * BL], F16, tag="xz",
                                   name=f"xz_t{t}_{ci}")
                    ks = ci * KH * 2 * 128
                    nc.sync.dma_start(
                        out=xz_t[:].rearrange("p (k two b) -> p k two b",
                                              two=2, b=BL),
                        in_=xz_d[t:t + 1, ks:ks + KH * 2 * 128].rearrange(
                            "o (k p two) b -> p (o k) two b", p=128, two=2))
                    tiles[ci] = xz_t

            def dma_w0(ci):
                for kk in range(KH):
                    k = ci * KH + kk
                    for sfx in ("a", "l"):
                        tl = w_sb["w0" + sfx][ci]
                        wd = w0a_d if sfx == "a" else w0l_d
                        nc.sync.dma_start(out=tl[:, kk * H0:(kk + 1) * H0],
                                          in_=wd[k * 128:(k + 1) * 128, :])

            def dma_w1d(ks):
                for k in ks:
                    nc.sync.dma_start(
                        out=w1d[:, k * NDIG * H1:(k + 1) * NDIG * H1],
                        in_=w1d_d[k * 128:(k + 1) * 128, :])

            def dma_wo():
                for k in range(KT2):
                    nc.sync.dma_start(out=woa[:, k * OUT:(k + 1) * OUT],
                                      in_=woa_d[k * 128:(k + 1) * 128, :])
                    nc.sync.dma_start(out=wol[:, k * OUT:(k + 1) * OUT],
                                      in_=wol_d[k * 128:(k + 1) * 128, :])

            # just-in-time DMA order (single serialized DMA resource)
            nc.sync.dma_start(out=cpos[:], in_=cst_d["cpos"][:])
            for l in (0, 1, 2):
                nc.sync.dma_start(out=cneg[l][:], in_=cst_d[f"cneg{l}"][:])
            load_x(0, cis=(0,))
            dma_w0(0)
            for nm in ("br0", "br1", "br2"):
                nc.sync.dma_start(out=b_sb[nm][:], in_=br_d[nm][:])
            dma_w1d(range(0, 2))
            load_x(0, cis=(1,))
            dma_w0(1)

            # init states + consts
            for l in (0, 1, 2):
                for nm in ("v0", "u0"):
                    nc.vector.memset(st[(l, nm)][:], 0.0)
            nc.vector.memset(c021[:], 0.021)
            nc.vector.memset(acc[:], 0.0)

            bias_of = {0: "br0", 1: "br1", 2: "br2"}

            def build_B2b():
                # B2b_l[p, (c,b)] = 2*b_l[c*128+p]: PE outer products, 4
                # chunks per pB fill, one batched copy per fill
                for l in (0, 1, 2):
                    for m0 in range(0, HS[l] // 128, 4):
                        mn = min(4, HS[l] // 128 - m0)
                        for j in range(mn):
                            m = m0 + j
                            nc.tensor.matmul(
                                pB[:, j * 128:(j + 1) * 128],
                                b_sb[bias_of[l]][:, m * 128:(m + 1) * 128],
                                cpos[:], start=True, stop=True,
                                skip_group_check=True)
                        nc.scalar.copy(
                            B2b[l][:, m0 * 128:(m0 + mn) * 128],
                            pB[:, :mn * 128])

            def inject_bias(l):
                # add -b*2^EL[l] into each C[l] 128-chunk at t=0
                for m in range(HS[l] // 128):
                    nc.tensor.matmul(
                        C[l][:, m * 128:(m + 1) * 128],
                        b_sb[bias_of[l]][:, m * 128:(m + 1) * 128],
                        cneg[l][:], start=False, stop=False,
                        skip_group_check=True)

            # ---- L0 matmuls (fp16 3-term; stationary = w0 chunks) ----
            def emit_L0(t, cis):
                load_x(t, cis=cis)
                tiles = x_pre[t]
                for ci in cis:
                    xz_t = tiles.pop(ci)
                    if not tiles:
                        x_pre.pop(t, None)
                    wa = w_sb["w0a"][ci]
                    wl = w_sb["w0l"][ci]
                    # C0 main terms first: the step's first matmuls only wait
                    # on the C0 release-stt, not the C0b one
                    for k in range(KH):
                        kg = ci * KH + k
                        ra = xz_t[:, (2 * k) * BL:(2 * k + 1) * BL]
                        for m in range(MT0):
                            first = (t == 0 and kg == 0 and
                                     (m * 128) % ZR == 0)
                            lwa = wa[:, k * H0 + m * 128:
                                     k * H0 + (m + 1) * 128]
                            nc.tensor.matmul(C[0][:, m * 128:(m + 1) * 128],
                                             lwa, ra, start=first, stop=False,
                                             skip_group_check=True)
                    for k in range(KH):
                        kg = ci * KH + k
                        ra = xz_t[:, (2 * k) * BL:(2 * k + 1) * BL]
                        rl = xz_t[:, (2 * k + 1) * BL:(2 * k + 2) * BL]
                        for m in range(MT0):
                            first = (t == 0 and kg == 0 and
                                     (m * 128) % ZR == 0)
                            last = (t == T - 1 and kg == KT0 - 1)
                            lwa = wa[:, k * H0 + m * 128:
                                     k * H0 + (m + 1) * 128]
                            lwl = wl[:, k * H0 + m * 128:
                                     k * H0 + (m + 1) * 128]
                            psb = C0b[:, m * 128:(m + 1) * 128]
                            nc.tensor.matmul(psb, lwa, rl, start=first,
                                             stop=False,
                                             skip_group_check=True)
                            nc.tensor.matmul(psb, lwl, ra, start=False,
                                             stop=last,
                                             skip_group_check=True)
                    if t == 0 and ci == NX0 - 1:
                        inject_bias(0)

            # ---- L1: fp8 DoubleRow digit matmuls ----
            def emit_dr(t):
                sP = sP_cur[0]
                for k in range(KT1):
                    for m in range(MT1):
                        for pi, (pa, pb_) in enumerate(PLANE_OF_PAIR):
                            first = (t == 0 and k == 0 and pi == 0 and
                                     (m * 128) % ZR == 0)
                            last = (t == T - 1 and k == KT1 - 1 and pi == 2)
                            base = (k * NDIG + 2 * pi) * H1
                            lhs = w1d[:, base:base + 2 * H1].rearrange(
                                "p (two h) -> p two h", two=2)[
                                :, :, m * 128:(m + 1) * 128]
                            rhs = sP[:, pa * H0:(pa + 2) * H0].rearrange(
                                "p (two h) -> p two h", two=2)[
                                :, :, k * 128:(k + 1) * 128]
                            nc.tensor.matmul(
                                C[1][:, m * 128:(m + 1) * 128], lhs, rhs,
                                start=first, stop=last, perf_mode=DR,
                                skip_group_check=True)
                if t == 0:
                    inject_bias(1)

            # ---- fused LIF elementwise (layout-agnostic) ----
            def lif_pre(l, t):
                """Off-path: A = v0*v0 - u0 + B2b (ACT square + 2 DVE ops)."""
                h = HS[l]
                A = A_[l][:, :h]
                v0, u0 = st[(l, "v0")], st[(l, "u0")]
                nc.scalar.square(A, v0[:])
                nc.vector.tensor_tensor(out=A, in0=A, in1=u0[:],
                                        op=Alu.subtract)
                nc.vector.tensor_tensor(out=A, in0=A, in1=B2b[l][:], op=Alu.add)

            def lif_release(l, t, halves=1):
                """DVE, reads PSUM: v = C*2^(-t-e) + A (+ C0b part for l=0).
                halves=2 on the last step (no filler work to hide the chain)."""
                h = HS[l]
                hh = h // halves
                for off in range(0, h, hh):
                    v = vT[l][:, off:off + hh]
                    nc.vector.scalar_tensor_tensor(
                        out=v, in0=C[l][:, off:off + hh],
                        scalar=float(2.0 ** (-t - EL[l])),
                        in1=A_[l][:, off:off + hh], op0=Alu.mult, op1=Alu.add)
                    if l == 0:
                        nc.vector.scalar_tensor_tensor(
                            out=v, in0=C0b[:, off:off + hh],
                            scalar=float(2.0 ** -(t + 11)),
                            in1=v, op0=Alu.mult, op1=Alu.add)

            def lif_spike(l, t, s_out, off, hh):
                """Spike threshold for one half (chain-critical)."""
                s_scale = 1.0 if l == 2 else float(2.0 ** t)
                nc.vector.tensor_scalar(
                    out=s_out[:, off:off + hh], in0=vT[l][:, off:off + hh],
                    scalar1=0.5, scalar2=s_scale, op0=Alu.is_gt, op1=Alu.mult)

            def lif_states(l, t, s_out, last):
                """State updates for step t+1 (off critical path)."""
                h = HS[l]
                v = vT[l][:, :h]
                v0, u0 = st[(l, "v0")], st[(l, "u0")]
                s_scale = 1.0 if l == 2 else float(2.0 ** t)
                if last:
                    return
                U = U_[l][:, :h]
                nc.vector.scalar_tensor_tensor(
                    out=U, in0=v0[:], scalar=float(-0.172 / 1.529), in1=u0[:],
                    op0=Alu.mult, op1=Alu.add)
                nc.scalar.mul(U, U, 1.529)
                nc.vector.scalar_tensor_tensor(
                    out=u0[:], in0=s_out[:], scalar=float(0.132 / s_scale),
                    in1=U, op0=Alu.mult, op1=Alu.add)
                nc.scalar.copy(v0[:], v)
                nc.vector.copy_predicated(out=v0[:],
                                          mask=s_out[:].bitcast(dt.uint16),
                                          data=c021[:, :h])

            def make_planes_half(s0, sP, off, hh):
                """4 fp8e5 scaled copies of one half of the L0 spikes."""
                nc.scalar.copy(sP[:, off:off + hh], s0[:, off:off + hh])
                nc.vector.tensor_scalar(
                    out=sP[:, H0 + off:H0 + off + hh], in0=s0[:, off:off + hh],
                    scalar1=float(2.0 ** -8), scalar2=None, op0=Alu.mult)
                nc.scalar.mul(sP[:, 2 * H0 + off:2 * H0 + off + hh],
                              s0[:, off:off + hh], float(2.0 ** -16))
                nc.scalar.mul(sP[:, 3 * H0 + off:3 * H0 + off + hh],
                              s0[:, off:off + hh], float(2.0 ** -16))

            sP_cur = [None]

            def emit_rest(t, filler=None):
                last = (t == T - 1)
                s0 = kp.tile([128, H0], F16, tag="s0", name=f"s0_t{t}")
                sP = kp.tile([128, 4 * H0], E5, tag="sP0", name=f"sP0_t{t}")
                sP_cur[0] = sP
                if last:
                    for off in (0, H0 // 2):
                        lif_spike(0, t, s0, off, H0 // 2)
                        make_planes_half(s0, sP, off, H0 // 2)
                else:
                    lif_spike(0, t, s0, 0, H0)
                    lif_states(0, t, s0, last)
                    lif_pre(0, t + 1)
                    make_planes_half(s0, sP, 0, H0)
                emit_dr(t)
                lif_release(1, t, halves=2 if last else 1)
                if filler is not None:
                    filler()
                s1 = kp.tile([128, H1], F16, tag="s1", name=f"s1_t{t}")
                s1L = kp.tile([128, H1], F16, tag="s1L", name=f"s1L_t{t}")
                if last:
                    for off in (0, H1 // 2):
                        lif_spike(1, t, s1, off, H1 // 2)
                        nc.scalar.mul(s1L[:, off:off + H1 // 2],
                                      s1[:, off:off + H1 // 2],
                                      float(2.0 ** -11))
                else:
                    lif_spike(1, t, s1, 0, H1)
                    nc.scalar.mul(s1L[:], s1[:], float(2.0 ** -11))
                    lif_states(1, t, s1, last)
                    lif_pre(1, t + 1)
                # L2: fp16 hi/lo 2-term (stationary = wo chunks); all hi terms
                # first so the PE has work before s1L lands
                for term in (0, 1):
                    for k in range(KT2):
                        ra = s1[:, k * 128:(k + 1) * 128]
                        rl = s1L[:, k * 128:(k + 1) * 128]
                        for m in range(MT2):
                            first = (term == 0 and t == 0 and k == 0 and
                                     (m * 128) % ZR == 0)
                            lastm = (term == 1 and t == T - 1 and k == KT2 - 1)
                            lwa = woa[:, k * OUT + m * 128:
                                      k * OUT + (m + 1) * 128]
                            lwl = wol[:, k * OUT + m * 128:
                                      k * OUT + (m + 1) * 128]
                            ps = C[2][:, m * 128:(m + 1) * 128]
                            if term == 0:
                                nc.tensor.matmul(ps, lwa, ra, start=first,
                                                 stop=False,
                                                 skip_group_check=True)
                            else:
                                nc.tensor.matmul(ps, lwl, rl, start=False,
                                                 stop=lastm,
                                                 skip_group_check=True)
                if t == 0:
                    inject_bias(2)
                lif_release(2, t)
                nc.vector.scalar_tensor_tensor(
                    out=acc[:], in0=vT[2][:, :OUT], scalar=0.5, in1=acc[:],
                    op0=Alu.is_gt, op1=Alu.add)
                if not last:
                    s2 = kp.tile([128, OUT], F16, tag="s2", name=f"s2_t{t}")
                    lif_spike(2, t, s2, 0, OUT)
                    lif_states(2, t, s2, last)
                    lif_pre(2, t + 1)

            # ---- main loop: 1-step layer skew ----
            for t in range(T):
                if t >= 1:
                    lif_release(0, t - 1)   # frees C0/C0b for step t's matmuls
                emit_L0(t, cis=(0,))
                if t == 0:
                    load_x(1, cis=(0,))
                    dma_w1d(range(2, 5))
                    load_x(1, cis=(1,))
                    dma_w1d(range(5, KT1))
                    dma_wo()
                    build_B2b()
                    for l in (0, 1, 2):
                        lif_pre(l, 0)
                    emit_L0(0, cis=(1,))
                else:
                    emit_rest(t - 1, filler=lambda tt=t: emit_L0(tt, cis=(1,)))
                    if t + 1 < T:
                        load_x(t + 1)
            lif_release(0, T - 1, halves=2)
            emit_rest(T - 1)

            nc.sync.dma_start(out=out_d[:], in_=acc[:])

    nc.compile()
    _BUILD_CACHE[key] = nc
    return nc


def _split_f16(a32, lo_scale=2048.0):
    hi = a32.astype(np.float16)
    lo = ((a32 - hi.astype(np.float32)) * np.float32(lo_scale)).astype(np.float16)
    return hi, lo


def _digit_planes(WT, ndig=NDIG, ew=EW):
    """WT [in,out] fp32 -> [in, ndig*out] fp8e4 digit planes (folded)."""
    r = WT.astype(np.float64) * (2.0 ** ew)
    assert np.max(np.abs(r)) <= 0.5, "weight exponent EW too small"
    planes = []
    for i in range(1, ndig + 1):
        di = np.rint(r * 16.0 ** i)
        di = np.clip(di, -4, 4) if i == ndig else np.clip(di, -8, 8)
        r = r - di * 16.0 ** -i
        planes.append(di * 2.0 ** -WFOLD[i - 1])
    out = np.concatenate(planes, axis=1).astype(ml_dtypes.float8_e4m3fn)
    assert np.all(out.astype(np.float64) == np.concatenate(planes, axis=1)), \
        "digit planes not exact in fp8e4"
    return out


def prep_inputs(in_pop_spikes, W0, b0, W1, b1, Wout, bout,
                T=16, BL=128, ncores=NCORES):
    x = np.ascontiguousarray(np.transpose(np.asarray(in_pop_spikes, np.float32),
                                          (2, 1, 0)))  # [T, IN, B]
    B = x.shape[2]
    IN = x.shape[1]
    scale = (2.0 ** np.arange(T, dtype=np.float32)).reshape(T, 1, 1)
    xh32 = x.astype(np.float16).astype(np.float32)
    xa = (xh32 * scale).astype(np.float16)
    xr = ((x - xh32) * (scale * np.float32(2048.0))).astype(np.float16)
    xz = np.stack([xa.reshape(T, IN // 128, 128, B),
                   xr.reshape(T, IN // 128, 128, B)], axis=3)
    xz = np.ascontiguousarray(xz.reshape(T, IN * 2, B))

    com = {}
    W0T = np.ascontiguousarray(np.asarray(W0, np.float32).T)
    com["w0a"], com["w0l"] = _split_f16(W0T)
    com["w1d"] = _digit_planes(np.ascontiguousarray(np.asarray(W1, np.float32).T))
    WoT = np.ascontiguousarray(np.asarray(Wout, np.float32).T)
    com["woa"], com["wol"] = _split_f16(WoT)
    for nm, b in (("br0", b0), ("br1", b1), ("br2", bout)):
        hi, lo = _split_f16(np.asarray(b, np.float32))
        com[nm] = np.stack([hi, lo])
    com["cpos"] = np.stack([np.full(128, 2.0, np.float16),
                            np.full(128, 2.0 / 2048.0, np.float16)])
    for l in (0, 1, 2):
        e = 2.0 ** EW if l == 1 else 1.0
        com[f"cneg{l}"] = np.stack([np.full(128, -e, np.float16),
                                    np.full(128, -e / 2048.0, np.float16)])

    in_maps = []
    for c in range(ncores):
        m = dict(com)
        m["xz"] = np.ascontiguousarray(xz[:, :, c * BL:(c + 1) * BL])
        in_maps.append(m)
    return in_maps


def kernel(in_pop_spikes, W0, b0, W1, b1, Wout, bout, batch_size, _trace=False):
    T = in_pop_spikes.shape[2]
    OUT, BL = Wout.shape[0], 128
    nc = build(**FULL)
    in_maps = prep_inputs(in_pop_spikes, W0, b0, W1, b1, Wout, bout, T=T)
    res = run_bass_kernel_spmd(nc, in_maps, core_ids=list(range(NCORES)),
                               trace=_trace)
    # device out[p, (c,b)] -> [b, c*128+p]
    outs = []
    for r in res.results:
        a = r["out"].reshape(128, OUT // 128, BL)
        outs.append(np.transpose(a, (2, 1, 0)).reshape(BL, OUT))
    out = (np.concatenate(outs, axis=0) / np.float32(T)).astype(np.float32)
    if _trace:
        kernel._last_results = res
    return out
